# revision 13
# baseline (speedup 1.0000x reference)
"""CFD-GCN Trainium2 kernel: 6-layer GCN + KNN-interpolate on 8 NeuronCores.

v3 strategy (node sharding, feature-major residency, bf16 sparse path):
  - Fine nodes sharded 6250/core (padded 6272 = 49*128 = nt tiles).
  - Per GCN layer: z = h @ W (dense, bf16, PE) kept node-major in SBUF
    (znm) AND written to DRAM shards zshA/zshB split by local row range
    (A = rows 0:3200 / tiles 0-24, B = rows 3200:6272 / tiles 25-48).
    The dense matmuls for layer l+1 are interleaved per-tile into the
    sparse phase of layer l, so AllGather-A fires ~halfway through the
    sparse phase and AllGather-B at its end -- both mostly hidden.
    Tables zfullA [8*3200, 512] / zfullB [8*3072, 512] keep row spaces
    within int16 gather-index range.
  - Edge gather: one dma_gather per (chunk, group of 2 dest tiles) with
    compile-time num_idxs (per-tile slots = max-over-cores count padded
    to x128 with dummy idx 0), no count registers.
  - Scatter-add: one-hot S blocks precomputed on the HOST as dense bf16
    [128 x 128] blocks (layout per tile [selfloop | A | B]) streamed
    from DRAM -- no on-device S construction. Self-loops are not in the
    edge lists: the diagonal selfloop block multiplies the SBUF-resident
    node-major z (dinv^2 * z) at zero gather cost.
  - pre0 (A h0 then W0) and end2 (W5 then A z5) run the same sparse
    machinery in bf16 against 128-wide tables h0A/h0B, z5fullA/z5fullB,
    reusing the same index tables and S blocks.
  - KNN-interpolate: matmul d2, DVE max8/max_index, small indirect
    gathers of coarse_y; overlaps the pre0 phase.
"""

import math
import numpy as np

# ---------------------------------------------------------------- constants
N_FINE = 50000
N_COARSE = 2000
HID = 512
OUT = 3
NCORES = 8
P = 128
CHA = 3200          # chunk A local rows (tiles 0..24)
CHB = 3072          # chunk B local rows (tiles 25..48)
TILES_A = CHA // P  # 25
GRP = 2             # dest tiles per gather group

_PROGRAM_CACHE = {}


# ---------------------------------------------------------------- host side
def _wrap16(flat, P=128):
    L = len(flat) // 16
    w = np.asarray(flat, np.int16).reshape(L, 16).T  # [16, L]
    return np.tile(w, (P // 16, 1))


def _preprocess_edges(edge_index, n_fine, ncores):
    """Dest-sorted edge lists split by source chunk + host-built S blocks."""
    import ml_dtypes
    bf16 = ml_dtypes.bfloat16
    nsh = n_fine // ncores              # 6250
    nt = math.ceil(nsh / P)             # 49
    padsh = nt * P                      # 6272

    row = np.asarray(edge_index[0]).astype(np.int64)
    col = np.asarray(edge_index[1]).astype(np.int64)

    deg = (np.bincount(col, minlength=n_fine) + 1.0).astype(np.float32)
    dinv = 1.0 / np.sqrt(deg)
    normv = (dinv[row] * dinv[col]).astype(np.float32)
    dinv2 = (dinv * dinv).astype(np.float32)

    order = np.argsort(col, kind="stable")
    col_s, row_s, norm_s = col[order], row[order], normv[order]

    src_core = row_s // nsh
    src_ls = row_s % nsh
    isa = src_ls < CHA
    idxA_val = src_core * CHA + src_ls
    idxB_val = src_core * CHB + (src_ls - CHA)

    cnt = np.zeros((ncores, nt, 2), np.int64)
    bounds = {}
    for c in range(ncores):
        base = c * nsh
        for t in range(nt):
            lo, hi = base + t * P, min(base + (t + 1) * P, base + nsh)
            a = np.searchsorted(col_s, lo, "left")
            b = np.searchsorted(col_s, hi, "left")
            na = int(isa[a:b].sum())
            cnt[c, t, 0] = na
            cnt[c, t, 1] = (b - a) - na
            bounds[(c, t)] = (a, b)

    KA = [int(math.ceil(max(1, cnt[:, t, 0].max()) / P)) for t in range(nt)]
    KB = [int(math.ceil(max(1, cnt[:, t, 1].max()) / P)) for t in range(nt)]
    totKA, totKB = sum(KA), sum(KB)
    offA = np.concatenate([[0], np.cumsum(KA)]) * P
    offB = np.concatenate([[0], np.cumsum(KB)]) * P
    colbase = [t + (offA[t] + offB[t]) // P for t in range(nt)]
    nblk = nt + totKA + totKB

    dvec = np.arange(P, dtype=np.float32)
    out = []
    for c in range(ncores):
        flatA = np.zeros(totKA * P, np.int64)
        flatB = np.zeros(totKB * P, np.int64)
        ecol = np.full((P, nblk), -1.0, np.float32)
        enorm = np.zeros((P, nblk), np.float32)
        base = c * nsh
        for t in range(nt):
            a, b = bounds[(c, t)]
            m = isa[a:b]
            crel = (col_s[a:b] - (base + t * P)).astype(np.float32)
            nrm = norm_s[a:b]
            cb = colbase[t]
            nvalid = min(nsh - t * P, P)
            pp = np.arange(nvalid)
            ecol[pp, cb] = pp
            enorm[pp, cb] = dinv2[base + t * P: base + t * P + nvalid]
            for half, (ids, off, flat, bcol) in enumerate((
                    (idxA_val[a:b][m], offA[t], flatA, cb + 1),
                    (idxB_val[a:b][~m], offB[t], flatB, cb + 1 + KA[t]))):
                n = len(ids)
                flat[off: off + n] = ids
                s = np.arange(n)
                cc = crel[m] if half == 0 else crel[~m]
                nn = nrm[m] if half == 0 else nrm[~m]
                ecol[s % P, bcol + s // P] = cc
                enorm[s % P, bcol + s // P] = nn
        sblk = ((ecol[:, :, None] == dvec[None, None, :])
                * enorm[:, :, None]).astype(bf16).reshape(P, nblk * P)
        out.append({
            "idxA": _wrap16(flatA), "idxB": _wrap16(flatB),
            "sblk": sblk,
        })
    return KA, KB, nt, padsh, out


# ---------------------------------------------------------------- device side
def build_program(n_fine, n_coarse, hid, out_dim, ncores, KA, KB, nt):
    import concourse.bass as bass
    import concourse.mybir as mybir
    from concourse.bacc import Bacc
    from concourse.tile import TileContext
    from concourse.masks import make_identity
    from contextlib import ExitStack

    F32 = mybir.dt.float32
    BF16 = mybir.dt.bfloat16
    I16 = mybir.dt.int16
    padsh = nt * P
    kc = hid // P
    rg = [list(range(ncores))]
    AF = mybir.ActivationFunctionType
    ALU = mybir.AluOpType
    IOO = bass.IndirectOffsetOnAxis
    ncpad = math.ceil(n_coarse / 512) * 512
    ncc = math.ceil(n_coarse / 512)

    totKA, totKB = sum(KA), sum(KB)
    offA = [0]
    offB = [0]
    for t in range(nt):
        offA.append(offA[-1] + KA[t] * P)
        offB.append(offB[-1] + KB[t] * P)
    colbase = [t + (offA[t] + offB[t]) // P for t in range(nt)]
    nblk = nt + totKA + totKB
    rowsA, rowsB = ncores * CHA, ncores * CHB

    groups = []
    t0 = 0
    while t0 < nt:
        t1 = min(t0 + GRP, nt)
        groups.append((t0, t1))
        t0 = t1
    GA = max(offA[t1] - offA[t0] for t0, t1 in groups) // P
    GB = max(offB[t1] - offB[t0] for t0, t1 in groups) // P

    nc = Bacc(num_devices=ncores)

    # ---- kernel I/O (per core) ----
    h0A = nc.declare_dram_parameter("h0A", [rowsA, P], BF16, isOutput=False)
    h0B = nc.declare_dram_parameter("h0B", [rowsB, P], BF16, isOutput=False)
    h0nm_d = nc.declare_dram_parameter("h0nm", [padsh, 8], BF16, isOutput=False)
    idxA = nc.declare_dram_parameter("idxA", [P, totKA * 8], I16, isOutput=False)
    idxB = nc.declare_dram_parameter("idxB", [P, totKB * 8], I16, isOutput=False)
    sblk = nc.declare_dram_parameter("sblk", [P, nblk * P], BF16, isOutput=False)
    xposT = nc.declare_dram_parameter("xposT", [2, padsh], F32, isOutput=False)
    xpos_nm = nc.declare_dram_parameter("xpos_nm", [padsh, 2], F32, isOutput=False)
    coarseT = nc.declare_dram_parameter("coarseT", [2, n_coarse], F32, isOutput=False)
    ycoarse = nc.declare_dram_parameter("ycoarse", [n_coarse, out_dim], F32, isOutput=False)
    w_mid = [nc.declare_dram_parameter(n, [hid, hid], F32, isOutput=False)
             for n in ("w1", "w2", "we0", "we1")]
    b_mid = [nc.declare_dram_parameter(n, [hid], F32, isOutput=False)
             for n in ("b1", "b2", "be0", "be1")]
    w0 = nc.declare_dram_parameter("w0", [6, hid], F32, isOutput=False)
    b0 = nc.declare_dram_parameter("b0", [hid], F32, isOutput=False)
    wtop = nc.declare_dram_parameter("wtop", [out_dim, hid], F32, isOutput=False)
    w5 = nc.declare_dram_parameter("w5", [hid, out_dim], F32, isOutput=False)
    b5 = nc.declare_dram_parameter("b5", [out_dim], F32, isOutput=False)
    y_out = nc.declare_dram_parameter("out", [padsh, out_dim], F32, isOutput=True)

    # ---- internal DRAM ----
    zshA = [nc.dram_tensor(f"zshA{i}", [CHA, hid], BF16) for i in range(4)]
    zshB = [nc.dram_tensor(f"zshB{i}", [CHB, hid], BF16) for i in range(4)]
    zfullA = [nc.dram_tensor(f"zfullA{i}", [rowsA, hid], BF16, addr_space="Shared")
              for i in range(4)]
    zfullB = [nc.dram_tensor(f"zfullB{i}", [rowsB, hid], BF16, addr_space="Shared")
              for i in range(4)]
    z5shA = nc.dram_tensor("z5shA", [CHA, P], BF16)
    z5shB = nc.dram_tensor("z5shB", [CHB, P], BF16)
    z5fullA = nc.dram_tensor("z5fullA", [rowsA, P], BF16, addr_space="Shared")
    z5fullB = nc.dram_tensor("z5fullB", [rowsB, P], BF16, addr_space="Shared")

    with TileContext(nc) as tc:
        with ExitStack() as ctx:
            main = ctx.enter_context(tc.tile_pool(name="main", bufs=1))
            sp = ctx.enter_context(tc.tile_pool(name="sp", bufs=3))
            mp = ctx.enter_context(tc.tile_pool(name="mp", bufs=2))
            smallp = ctx.enter_context(tc.tile_pool(name="smallp", bufs=2))
            ppA = ctx.enter_context(tc.tile_pool(name="ppA", bufs=2, space="PSUM"))
            ppB = ctx.enter_context(tc.tile_pool(name="ppB", bufs=2, space="PSUM"))
            ppC = ctx.enter_context(tc.tile_pool(name="ppC", bufs=2, space="PSUM"))

            def accps(shape):
                return ppA.tile(shape, F32, tag="acc", name="acc")

            def densps(shape):
                return ppB.tile(shape, F32, tag="dacc", name="dacc")

            def tps(shape):
                return ppC.tile(shape, F32, tag="tp", name="tp")

            # ---------- persistent tiles ----------
            hT = main.tile([P, kc, padsh], BF16, tag="hT")
            znm = main.tile([P, nt, hid], BF16, tag="znm")
            z5nm = main.tile([P, nt, 4], BF16, tag="z5nm")
            h0nm = main.tile([P, nt, 8], BF16, tag="h0nm")
            y3n = main.tile([P, nt, out_dim], F32, tag="y3n")
            iden = main.tile([P, P], F32, tag="iden")
            idxA_sb = main.tile([P, totKA * 8], I16, tag="idxA_sb")
            idxB_sb = main.tile([P, totKB * 8], I16, tag="idxB_sb")
            wtop_sb = main.tile([out_dim, hid], F32, tag="wtop_sb")
            w0_sb = main.tile([6, hid], F32, tag="w0_sb")
            w_sb = [main.tile([P, kc, hid], BF16, tag=f"w_sb{i}", name=f"w_sb{i}")
                    for i in range(4)]
            b_sb = [main.tile([P, kc], F32, tag=f"b_sb{i}", name=f"b_sb{i}")
                    for i in range(4)]
            b0_sb = main.tile([P, kc], F32, tag="b0_sb")
            w5_sb = main.tile([P, kc, out_dim], BF16, tag="w5_sb")
            b5_sb = main.tile([out_dim, 1], F32, tag="b5_sb")

            nc.sync.dma_start(out=idxA_sb[:], in_=idxA[:, :])
            nc.sync.dma_start(out=idxB_sb[:], in_=idxB[:, :])
            nc.sync.dma_start(out=wtop_sb[:], in_=wtop[:, :])
            nc.sync.dma_start(out=w0_sb[:], in_=w0[:, :])
            nc.sync.dma_start(
                out=h0nm[:], in_=h0nm_d[:, :].rearrange("(t p) d -> p t d", p=P))
            for i in range(4):
                nc.gpsimd.dma_start(
                    out=w_sb[i][:],
                    in_=w_mid[i][:, :].rearrange("(k p) h -> p k h", p=P))
                nc.sync.dma_start(
                    out=b_sb[i][:], in_=b_mid[i][:].rearrange("(k p) -> p k", p=P))
            nc.sync.dma_start(out=b0_sb[:], in_=b0[:].rearrange("(k p) -> p k", p=P))
            nc.gpsimd.dma_start(
                out=w5_sb[:], in_=w5[:, :].rearrange("(k p) o -> p k o", p=P))
            nc.sync.dma_start(out=b5_sb[:], in_=b5[:, None])
            make_identity(nc, iden[:])

            # ---------- helpers ----------
            def gather_group(tabA, tabB, g0, g1, elem, tag):
                nbA = (offA[g1] - offA[g0]) // P
                nbB = (offB[g1] - offB[g0]) // P
                mA = mp.tile([P, GA, elem], BF16, tag=tag + "A", name=tag + "A")
                mB = mp.tile([P, GB, elem], BF16, tag=tag + "B", name=tag + "B")
                nc.gpsimd.dma_gather(
                    mA[:, 0:nbA, :], tabA,
                    idxA_sb[:, offA[g0] // 16: offA[g1] // 16],
                    nbA * P, nbA * P, elem)
                nc.gpsimd.dma_gather(
                    mB[:, 0:nbB, :], tabB,
                    idxB_sb[:, offB[g0] // 16: offB[g1] // 16],
                    nbB * P, nbB * P, elem)
                return mA, mB

            def load_S(t):
                nbt = 1 + KA[t] + KB[t]
                St = sp.tile([P, nbt * P], BF16, tag="St", name="St")
                nc.sync.dma_start(
                    out=St[:], in_=sblk[:, colbase[t] * P:(colbase[t] + nbt) * P])
                return St

            def scatter_tile(t, g0, mA, mB, self_lhsT, acc, nrows, cchunks, St):
                bA = (offA[t] - offA[g0]) // P
                bB = (offB[t] - offB[g0]) // P
                for c0 in cchunks:
                    out = (acc[0:nrows, c0 * P:c0 * P + P] if nrows < P
                           else acc[:, c0 * P:c0 * P + P])
                    nc.tensor.matmul(out=out, lhsT=self_lhsT(c0),
                                     rhs=St[:, 0:P], start=True, stop=False)
                    for j in range(KA[t]):
                        nc.tensor.matmul(
                            out=out,
                            lhsT=(mA[:, bA + j, c0 * P:c0 * P + P] if nrows == P
                                  else mA[:, bA + j, 0:nrows]),
                            rhs=St[:, (1 + j) * P:(2 + j) * P],
                            start=False, stop=False)
                    for j in range(KB[t]):
                        nc.tensor.matmul(
                            out=out,
                            lhsT=(mB[:, bB + j, c0 * P:c0 * P + P] if nrows == P
                                  else mB[:, bB + j, 0:nrows]),
                            rhs=St[:, (1 + KA[t] + j) * P:(2 + KA[t] + j) * P],
                            start=False, stop=(j == KB[t] - 1))

            def dense_tile(li, t):
                # z_{li} = h @ W_li for tile t -> znm + zsh; AGs fired at
                # chunk boundaries so they overlap the remaining sparse work
                tp_ = t * P
                zps = densps([P, hid])
                for k in range(kc):
                    nc.tensor.matmul(out=zps[:], lhsT=hT[:, k, tp_:tp_ + P],
                                     rhs=w_sb[li][:, k, :], start=(k == 0),
                                     stop=(k == kc - 1) and li != 2)
                if li == 2:
                    pt3 = tps([P, P])
                    nc.tensor.transpose(out=pt3[0:out_dim, 0:P],
                                        in_=y3n[:, t, :], identity=iden[:])
                    y3t_T = smallp.tile([out_dim, P], F32, tag="y3t_T")
                    nc.vector.tensor_copy(out=y3t_T[:], in_=pt3[0:out_dim, 0:P])
                    nc.tensor.matmul(out=zps[:], lhsT=y3t_T[:],
                                     rhs=wtop_sb[:, :], start=False, stop=True)
                nc.scalar.activation(out=znm[:, t, :], in_=zps[:], func=AF.Copy)
                if t < TILES_A:
                    nc.sync.dma_start(out=zshA[li][tp_:tp_ + P, :], in_=znm[:, t, :])
                else:
                    nc.sync.dma_start(out=zshB[li][tp_ - CHA:tp_ - CHA + P, :],
                                      in_=znm[:, t, :])
                if t == TILES_A - 1:
                    nc.gpsimd.collective_compute(
                        "AllGather", ALU.bypass, replica_groups=rg,
                        ins=[zshA[li][:, :]], outs=[zfullA[li][:, :]])
                if t == nt - 1:
                    nc.gpsimd.collective_compute(
                        "AllGather", ALU.bypass, replica_groups=rg,
                        ins=[zshB[li][:, :]], outs=[zfullB[li][:, :]])

            def z5dense_tile(t):
                tp_ = t * P
                z5ps = densps([P, hid])
                for k in range(kc):
                    nc.tensor.matmul(out=z5ps[0:out_dim, 0:P], lhsT=w5_sb[:, k, :],
                                     rhs=hT[:, k, tp_:tp_ + P], start=(k == 0),
                                     stop=(k == kc - 1))
                z5T_sb = smallp.tile([out_dim, P], F32, tag="z5T_sb")
                nc.vector.tensor_copy(out=z5T_sb[:], in_=z5ps[0:out_dim, 0:P])
                ptp = tps([P, P])
                nc.tensor.transpose(out=ptp[:, 0:out_dim], in_=z5T_sb[:],
                                    identity=iden[0:out_dim, 0:out_dim])
                nc.vector.tensor_copy(out=z5nm[:, t, 0:out_dim],
                                      in_=ptp[:, 0:out_dim])
                if t < TILES_A:
                    nc.sync.dma_start(out=z5shA[tp_:tp_ + P, 0:out_dim],
                                      in_=z5nm[:, t, 0:out_dim])
                else:
                    nc.sync.dma_start(out=z5shB[tp_ - CHA:tp_ - CHA + P, 0:out_dim],
                                      in_=z5nm[:, t, 0:out_dim])
                if t == TILES_A - 1:
                    nc.gpsimd.collective_compute(
                        "AllGather", ALU.bypass, replica_groups=rg,
                        ins=[z5shA[:, :]], outs=[z5fullA[:, :]])
                if t == nt - 1:
                    nc.gpsimd.collective_compute(
                        "AllGather", ALU.bypass, replica_groups=rg,
                        ins=[z5shB[:, :]], outs=[z5fullB[:, :]])

            # ---------- pre0: q = A h0 (6-wide), z0 = W0^T q, relu; dense0 ----
            for g0, g1 in groups:
                mA, mB = gather_group(h0A[:, :], h0B[:, :], g0, g1, P, "m6")
                for t in range(g0, g1):
                    tp_ = t * P
                    St = load_S(t)
                    q = accps([P, P])
                    scatter_tile(t, g0, mA, mB, lambda c0: h0nm[:, t, 0:6],
                                 q, 6, [0], St)
                    q_sb = smallp.tile([6, P], F32, tag="q_sb")
                    nc.vector.tensor_copy(out=q_sb[:], in_=q[0:6, 0:P])
                    for jj in range(kc):
                        z0 = densps([P, P])
                        nc.tensor.matmul(out=z0[:, 0:P],
                                         lhsT=w0_sb[:, jj * P:(jj + 1) * P],
                                         rhs=q_sb[:], start=True, stop=True)
                        nc.scalar.activation(out=hT[:, jj, tp_:tp_ + P],
                                             in_=z0[:, 0:P], func=AF.Relu,
                                             bias=b0_sb[:, jj:jj + 1])
                    dense_tile(0, t)

            # ---------- KNN (independent; writes y3n) ----------
            with tc.tile_pool(name="knn", bufs=2) as kp:
                mones_sb = kp.tile([1, P], F32, tag="mones_sb", bufs=1)
                nc.vector.memset(mones_sb[:], -1.0)
                coarse3 = kp.tile([3, n_coarse], F32, tag="coarse3", bufs=1)
                with tc.tile_pool(name="knnprep", bufs=1) as kprep:
                    nc.sync.dma_start(out=coarse3[0:2, :], in_=coarseT[:, :])
                    pones = kprep.tile([2, 1], F32, tag="pones")
                    nc.vector.memset(pones[:], 1.0)
                    for i in range(ncc):
                        a, b = i * 512, min((i + 1) * 512, n_coarse)
                        sqc = kprep.tile([2, 512], F32, tag="sqc")
                        nc.vector.tensor_tensor(out=sqc[:, : b - a],
                                                in0=coarse3[0:2, a:b],
                                                in1=coarse3[0:2, a:b], op=ALU.mult)
                        ps = tps([P, 512])
                        nc.tensor.matmul(out=ps[0:1, : b - a], lhsT=pones[:],
                                         rhs=sqc[:, : b - a], start=True, stop=True)
                        csq = kprep.tile([1, 512], F32, tag="csq")
                        nc.vector.tensor_copy(out=csq[:, : b - a],
                                              in_=ps[0:1, : b - a])
                        nc.sync.dma_start(out=coarse3[2:3, a:b],
                                          in_=csq[:, : b - a])

                    xnm = kprep.tile([P, nt, 2], F32, tag="xnm")
                    nc.sync.dma_start(
                        out=xnm[:], in_=xpos_nm[:, :].rearrange("(t p) d -> p t d", p=P))
                    sqn = kprep.tile([P, nt, 2], F32, tag="sqn")
                    nc.vector.tensor_tensor(out=sqn[:], in0=xnm[:], in1=xnm[:],
                                            op=ALU.mult)
                    fsqneg = kp.tile([P, nt], F32, tag="fsqneg", bufs=1)
                    nc.vector.tensor_reduce(out=fsqneg[:], in_=sqn[:],
                                            axis=mybir.AxisListType.X, op=ALU.add,
                                            negate=True)

                for t in range(nt):
                    tp_ = t * P
                    xp_t = kp.tile([2, P], F32, tag="xp_t")
                    nc.sync.dma_start(out=xp_t[:], in_=xposT[:, tp_:tp_ + P])
                    lhsT3 = kp.tile([3, P], F32, tag="lhsT3")
                    nc.vector.tensor_scalar_mul(lhsT3[0:2, :], xp_t[:], 2.0)
                    nc.sync.dma_start(out=lhsT3[2:3, :], in_=mones_sb[:])

                    d2 = kp.tile([P, ncpad], F32, tag="d2", bufs=1)
                    for i in range(ncc):
                        a, b = i * 512, min((i + 1) * 512, n_coarse)
                        dps = densps([P, 512])
                        nc.tensor.matmul(out=dps[:, : b - a], lhsT=lhsT3[:],
                                         rhs=coarse3[:, a:b], start=True, stop=True)
                        nc.vector.tensor_scalar(out=d2[:, a:b], in0=dps[:, : b - a],
                                                scalar1=fsqneg[:, t:t + 1],
                                                scalar2=None, op0=ALU.add)
                    vals = kp.tile([P, 8], F32, tag="vals")
                    nc.vector.max(out=vals[:], in_=d2[:, 0:n_coarse])
                    idxs = kp.tile([P, 8], mybir.dt.uint32, tag="idxs")
                    nc.vector.max_index(out=idxs[:], in_max=vals[:],
                                        in_values=d2[:, 0:n_coarse])
                    wv = kp.tile([P, 3], F32, tag="wv")
                    nc.vector.tensor_scalar(out=wv[:], in0=vals[:, 0:3],
                                            scalar1=-1.0, scalar2=1e-16,
                                            op0=ALU.mult, op1=ALU.max)
                    nc.vector.reciprocal(out=wv[:], in_=wv[:])
                    wsum = kp.tile([P, 1], F32, tag="wsum")
                    nc.vector.tensor_reduce(out=wsum[:], in_=wv[:],
                                            axis=mybir.AxisListType.X, op=ALU.add)
                    nc.vector.reciprocal(out=wsum[:], in_=wsum[:])
                    nc.vector.tensor_scalar(out=wv[:], in0=wv[:],
                                            scalar1=wsum[:, 0:1], scalar2=None,
                                            op0=ALU.mult)
                    yg = kp.tile([P, 3, out_dim], F32, tag="yg")
                    for k3 in range(3):
                        nc.gpsimd.indirect_dma_start(
                            out=yg[:, k3, :], out_offset=None, in_=ycoarse[:, :],
                            in_offset=IOO(ap=idxs[:, k3:k3 + 1], axis=0))
                    tmp = kp.tile([P, out_dim], F32, tag="tmp")
                    nc.vector.tensor_scalar(out=y3n[:, t, :], in0=yg[:, 0, :],
                                            scalar1=wv[:, 0:1], scalar2=None,
                                            op0=ALU.mult)
                    for k in (1, 2):
                        nc.vector.tensor_scalar(out=tmp[:], in0=yg[:, k, :],
                                                scalar1=wv[:, k:k + 1], scalar2=None,
                                                op0=ALU.mult)
                        nc.vector.tensor_tensor(out=y3n[:, t, :], in0=y3n[:, t, :],
                                                in1=tmp[:], op=ALU.add)

            # ---------- mid layers: sparse(li) + interleaved dense(li+1) ----
            for li in range(4):
                for g0, g1 in groups:
                    mA, mB = gather_group(zfullA[li][:, :], zfullB[li][:, :],
                                          g0, g1, hid, "mm")
                    for t in range(g0, g1):
                        tp_ = t * P
                        St = load_S(t)
                        hps = accps([P, kc * P])
                        scatter_tile(t, g0, mA, mB,
                                     lambda c0: znm[:, t, c0 * P:c0 * P + P],
                                     hps, P, list(range(kc)), St)
                        for cc in range(kc):
                            nc.scalar.activation(out=hT[:, cc, tp_:tp_ + P],
                                                 in_=hps[:, cc * P:(cc + 1) * P],
                                                 func=AF.Relu,
                                                 bias=b_sb[li][:, cc:cc + 1])
                        if li < 3:
                            dense_tile(li + 1, t)
                        else:
                            z5dense_tile(t)

            # ---------- end2 final: out = A z5 + b5 ----------
            for g0, g1 in groups:
                mA, mB = gather_group(z5fullA[:, :], z5fullB[:, :], g0, g1, P, "m6")
                for t in range(g0, g1):
                    tp_ = t * P
                    St = load_S(t)
                    ops = accps([P, P])
                    scatter_tile(t, g0, mA, mB, lambda c0: z5nm[:, t, 0:out_dim],
                                 ops, out_dim, [0], St)
                    oT = smallp.tile([out_dim, P], F32, tag="oT")
                    nc.vector.tensor_scalar(out=oT[:], in0=ops[0:out_dim, 0:P],
                                            scalar1=b5_sb[:, 0:1], scalar2=None,
                                            op0=ALU.add)
                    po = tps([P, P])
                    nc.tensor.transpose(out=po[:, 0:out_dim], in_=oT[:],
                                        identity=iden[0:out_dim, 0:out_dim])
                    o_sb = smallp.tile([P, out_dim], F32, tag="o_sb")
                    nc.vector.tensor_copy(out=o_sb[:], in_=po[:, 0:out_dim])
                    nc.sync.dma_start(out=y_out[tp_:tp_ + P, :], in_=o_sb[:])

    nc.finalize()
    return nc


# ---------------------------------------------------------------- entry point
def _prepare(inputs, n_fine, n_coarse, hid, out_dim, ncores):
    import ml_dtypes
    bf16 = ml_dtypes.bfloat16
    x = np.asarray(inputs["x"], np.float32)
    sdf = np.asarray(inputs["sdf"], np.float32)
    coarse_x = np.asarray(inputs["coarse_x"], np.float32)
    coarse_y = np.asarray(inputs["coarse_y"], np.float32)
    edge_index = np.asarray(inputs["edge_index"])

    KA, KB, nt, padsh, edges = _preprocess_edges(edge_index, n_fine, ncores)
    nsh = n_fine // ncores

    h0 = np.zeros((n_fine, P), np.float32)
    h0[:, 0:5] = x
    h0[:, 5:6] = sdf
    h0A = np.zeros((ncores * CHA, P), bf16)
    h0B = np.zeros((ncores * CHB, P), bf16)
    for c in range(ncores):
        sh = h0[c * nsh:(c + 1) * nsh]
        h0A[c * CHA:(c + 1) * CHA] = sh[:CHA].astype(bf16)
        h0B[c * CHB:c * CHB + (nsh - CHA)] = sh[CHA:].astype(bf16)

    xpos = x[:, :2].astype(np.float32)
    coarseT = np.ascontiguousarray(coarse_x[:, :2].T).astype(np.float32)

    in_maps = []
    for c in range(ncores):
        xx = np.zeros((2, padsh), np.float32)
        xx[:, :nsh] = xpos[c * nsh:(c + 1) * nsh].T
        xn = np.zeros((padsh, 2), np.float32)
        xn[:nsh] = xpos[c * nsh:(c + 1) * nsh]
        h0nm = np.zeros((padsh, 8), bf16)
        h0nm[:nsh, 0:6] = h0[c * nsh:(c + 1) * nsh, 0:6].astype(bf16)
        m = {
            "h0A": h0A, "h0B": h0B, "h0nm": h0nm,
            "idxA": edges[c]["idxA"], "idxB": edges[c]["idxB"],
            "sblk": edges[c]["sblk"],
            "xposT": xx, "xpos_nm": xn,
            "coarseT": coarseT, "ycoarse": coarse_y,
            "w0": np.asarray(inputs["pre_W0"], np.float32),
            "b0": np.asarray(inputs["pre_b0"], np.float32),
            "w1": np.asarray(inputs["pre_W1"], np.float32),
            "b1": np.asarray(inputs["pre_b1"], np.float32),
            "w2": np.asarray(inputs["pre_W2"], np.float32),
            "b2": np.asarray(inputs["pre_b2"], np.float32),
            "wtop": np.ascontiguousarray(np.asarray(inputs["end_W0"], np.float32)[:out_dim]),
            "we0": np.ascontiguousarray(np.asarray(inputs["end_W0"], np.float32)[out_dim:]),
            "be0": np.asarray(inputs["end_b0"], np.float32),
            "we1": np.asarray(inputs["end_W1"], np.float32),
            "be1": np.asarray(inputs["end_b1"], np.float32),
            "w5": np.asarray(inputs["end_W2"], np.float32),
            "b5": np.asarray(inputs["end_b2"], np.float32),
        }
        in_maps.append(m)
    return KA, KB, nt, padsh, in_maps


def run(inputs, n_fine=N_FINE, n_coarse=N_COARSE, hid=HID, out_dim=OUT,
        ncores=NCORES, sim=False, trace=False):
    KA, KB, nt, padsh, in_maps = _prepare(inputs, n_fine, n_coarse, hid,
                                          out_dim, ncores)
    key = (n_fine, n_coarse, hid, out_dim, ncores, tuple(KA), tuple(KB), nt)
    if key not in _PROGRAM_CACHE:
        _PROGRAM_CACHE[key] = build_program(n_fine, n_coarse, hid, out_dim,
                                            ncores, KA, KB, nt)
    nc = _PROGRAM_CACHE[key]

    nsh = n_fine // ncores
    if sim:
        from concourse.bass_interp import MultiCoreSim
        ms = MultiCoreSim(nc, ncores, num_workers=1)
        for c in range(ncores):
            for k, v in in_maps[c].items():
                ms.cores[c].tensor(k)[:] = v
        ms.simulate()
        outs = [np.array(ms.cores[c].tensor("out")) for c in range(ncores)]
        exec_ns = None
    else:
        from concourse.bass_utils import run_bass_kernel_spmd
        res = run_bass_kernel_spmd(nc, in_maps, list(range(ncores)), trace=trace)
        outs = [res.results[c]["out"] for c in range(ncores)]
        exec_ns = res.exec_time_ns

    full = np.zeros((n_fine, out_dim), np.float32)
    for c in range(ncores):
        full[c * nsh:(c + 1) * nsh] = outs[c][:nsh]
    return full, exec_ns


def kernel(**inputs):
    out, _ = run(inputs)
    return out


# revision 16
# speedup vs baseline: 1.1441x; 1.1441x over previous
"""CFD-GCN Trainium2 kernel: 6-layer GCN + KNN-interpolate on 8 NeuronCores.

v3 strategy (node sharding, feature-major residency, bf16 sparse path):
  - Fine nodes sharded 6250/core (padded 6272 = 49*128 = nt tiles).
  - Per GCN layer: z = h @ W (dense, bf16, PE) kept node-major in SBUF
    (znm) AND written to DRAM shards zshA/zshB split by local row range
    (A = rows 0:3200 / tiles 0-24, B = rows 3200:6272 / tiles 25-48).
    The dense matmuls for layer l+1 are interleaved per-tile into the
    sparse phase of layer l, so AllGather-A fires ~halfway through the
    sparse phase and AllGather-B at its end -- both mostly hidden.
    Tables zfullA [8*3200, 512] / zfullB [8*3072, 512] keep row spaces
    within int16 gather-index range.
  - Edge gather: one dma_gather per (chunk, group of 2 dest tiles) with
    compile-time num_idxs (per-tile slots = max-over-cores count padded
    to x128 with dummy idx 0), no count registers.
  - Scatter-add: one-hot S blocks precomputed on the HOST as dense bf16
    [128 x 128] blocks (layout per tile [selfloop | A | B]) streamed
    from DRAM -- no on-device S construction. Self-loops are not in the
    edge lists: the diagonal selfloop block multiplies the SBUF-resident
    node-major z (dinv^2 * z) at zero gather cost.
  - pre0 (A h0 then W0) and end2 (W5 then A z5) run the same sparse
    machinery in bf16 against 128-wide tables h0A/h0B, z5fullA/z5fullB,
    reusing the same index tables and S blocks.
  - KNN-interpolate: matmul d2, DVE max8/max_index, small indirect
    gathers of coarse_y; overlaps the pre0 phase.
"""

import math
import numpy as np

# ---------------------------------------------------------------- constants
N_FINE = 50000
N_COARSE = 2000
HID = 512
OUT = 3
NCORES = 8
P = 128
CHA = 3200          # chunk A local rows (tiles 0..24)
CHB = 3072          # chunk B local rows (tiles 25..48)
TILES_A = CHA // P  # 25
GRP = 2             # dest tiles per gather group

_PROGRAM_CACHE = {}


# ---------------------------------------------------------------- host side
def _wrap16(flat, P=128):
    L = len(flat) // 16
    w = np.asarray(flat, np.int16).reshape(L, 16).T  # [16, L]
    return np.tile(w, (P // 16, 1))


def _mk_groups(nt):
    groups = []
    t0 = 0
    while t0 < nt:
        groups.append((t0, min(t0 + GRP, nt)))
        t0 = min(t0 + GRP, nt)
    return groups


def _slot_layout(SL, nt):
    """Per-chunk layout: group-relative offsets, block spans, flat base."""
    groups = _mk_groups(nt)
    rel = [0] * nt
    slotbase = {}
    base = 0
    for g0, g1 in groups:
        r = 0
        for t in range(g0, g1):
            rel[t] = r
            r += SL[t]
        slotbase[g0] = base
        base += r
    bs = [rel[t] // P for t in range(nt)]
    be = [math.ceil((rel[t] + SL[t]) / P) for t in range(nt)]
    return groups, rel, slotbase, base, bs, be


def _preprocess_edges(edge_index, n_fine, ncores):
    """Dest-sorted edge lists split by source chunk + host-built S blocks.

    Slots are packed at 16 granularity (per tile: max-over-cores count
    rounded to x16); matmul blocks follow the group-relative 128 grid, so
    a block shared by two tiles gets two complementary masked S columns.
    """
    import ml_dtypes
    bf16 = ml_dtypes.bfloat16
    nsh = n_fine // ncores              # 6250
    nt = math.ceil(nsh / P)             # 49
    padsh = nt * P                      # 6272

    row = np.asarray(edge_index[0]).astype(np.int64)
    col = np.asarray(edge_index[1]).astype(np.int64)

    deg = (np.bincount(col, minlength=n_fine) + 1.0).astype(np.float32)
    dinv = 1.0 / np.sqrt(deg)
    normv = (dinv[row] * dinv[col]).astype(np.float32)
    dinv2 = (dinv * dinv).astype(np.float32)

    order = np.argsort(col, kind="stable")
    col_s, row_s, norm_s = col[order], row[order], normv[order]

    src_core = row_s // nsh
    src_ls = row_s % nsh
    isa = src_ls < CHA
    idxA_val = src_core * CHA + src_ls
    idxB_val = src_core * CHB + (src_ls - CHA)

    cnt = np.zeros((ncores, nt, 2), np.int64)
    bounds = {}
    for c in range(ncores):
        base = c * nsh
        for t in range(nt):
            lo, hi = base + t * P, min(base + (t + 1) * P, base + nsh)
            a = np.searchsorted(col_s, lo, "left")
            b = np.searchsorted(col_s, hi, "left")
            na = int(isa[a:b].sum())
            cnt[c, t, 0] = na
            cnt[c, t, 1] = (b - a) - na
            bounds[(c, t)] = (a, b)

    SLA = [16 * int(math.ceil(max(1, cnt[:, t, 0].max()) / 16)) for t in range(nt)]
    SLB = [16 * int(math.ceil(max(1, cnt[:, t, 1].max()) / 16)) for t in range(nt)]
    groups, relA, sbaseA, totA, bsA, beA = _slot_layout(SLA, nt)
    _, relB, sbaseB, totB, bsB, beB = _slot_layout(SLB, nt)
    g_of = {}
    for g0, g1 in groups:
        for t in range(g0, g1):
            g_of[t] = g0
    nsAv = [beA[t] - bsA[t] for t in range(nt)]
    nsBv = [beB[t] - bsB[t] for t in range(nt)]
    colbase = [0] * nt
    acc = 0
    for t in range(nt):
        colbase[t] = acc
        acc += 1 + nsAv[t] + nsBv[t]
    nblk = acc

    dvec = np.arange(P, dtype=np.float32)
    out = []
    for c in range(ncores):
        flatA = np.zeros(totA, np.int64)
        flatB = np.zeros(totB, np.int64)
        ecol = np.full((P, nblk), -1.0, np.float32)
        enorm = np.zeros((P, nblk), np.float32)
        base = c * nsh
        for t in range(nt):
            a, b = bounds[(c, t)]
            m = isa[a:b]
            crel = (col_s[a:b] - (base + t * P)).astype(np.float32)
            nrm = norm_s[a:b]
            cb = colbase[t]
            nvalid = min(nsh - t * P, P)
            pp = np.arange(nvalid)
            ecol[pp, cb] = pp
            enorm[pp, cb] = dinv2[base + t * P: base + t * P + nvalid]
            for ids, rel, sbase, bs, flat, bcol in (
                    (idxA_val[a:b][m], relA[t], sbaseA[g_of[t]], bsA[t],
                     flatA, cb + 1),
                    (idxB_val[a:b][~m], relB[t], sbaseB[g_of[t]], bsB[t],
                     flatB, cb + 1 + nsAv[t])):
                n = len(ids)
                flat[sbase + rel: sbase + rel + n] = ids
                q = rel + np.arange(n)
                cc = crel[m] if flat is flatA else crel[~m]
                nn = nrm[m] if flat is flatA else nrm[~m]
                ecol[q % P, bcol + q // P - bs] = cc
                enorm[q % P, bcol + q // P - bs] = nn
        sblk = ((ecol[:, :, None] == dvec[None, None, :])
                * enorm[:, :, None]).astype(bf16).reshape(P, nblk * P)
        out.append({
            "idxA": _wrap16(flatA), "idxB": _wrap16(flatB),
            "sblk": sblk,
        })
    return SLA, SLB, nt, padsh, out


# ---------------------------------------------------------------- device side
def build_program(n_fine, n_coarse, hid, out_dim, ncores, SLA, SLB, nt):
    import concourse.bass as bass
    import concourse.mybir as mybir
    from concourse.bacc import Bacc
    from concourse.tile import TileContext
    from concourse.masks import make_identity
    from contextlib import ExitStack

    F32 = mybir.dt.float32
    BF16 = mybir.dt.bfloat16
    I16 = mybir.dt.int16
    padsh = nt * P
    kc = hid // P
    rg = [list(range(ncores))]
    AF = mybir.ActivationFunctionType
    ALU = mybir.AluOpType
    IOO = bass.IndirectOffsetOnAxis
    ncpad = math.ceil(n_coarse / 512) * 512
    ncc = math.ceil(n_coarse / 512)

    groups, relA, sbaseA, totA, bsA, beA = _slot_layout(SLA, nt)
    _, relB, sbaseB, totB, bsB, beB = _slot_layout(SLB, nt)
    nsAv = [beA[t] - bsA[t] for t in range(nt)]
    nsBv = [beB[t] - bsB[t] for t in range(nt)]
    colbase = [0] * nt
    _acc = 0
    for t in range(nt):
        colbase[t] = _acc
        _acc += 1 + nsAv[t] + nsBv[t]
    nblk = _acc
    rowsA, rowsB = ncores * CHA, ncores * CHB
    GA = max(math.ceil(sum(SLA[t] for t in range(g0, g1)) / P)
             for g0, g1 in groups)
    GB = max(math.ceil(sum(SLB[t] for t in range(g0, g1)) / P)
             for g0, g1 in groups)

    nc = Bacc(num_devices=ncores)

    # ---- kernel I/O (per core) ----
    h0A = nc.declare_dram_parameter("h0A", [rowsA, P], BF16, isOutput=False)
    h0B = nc.declare_dram_parameter("h0B", [rowsB, P], BF16, isOutput=False)
    h0nm_d = nc.declare_dram_parameter("h0nm", [padsh, 8], BF16, isOutput=False)
    idxA = nc.declare_dram_parameter("idxA", [P, totA // 16], I16, isOutput=False)
    idxB = nc.declare_dram_parameter("idxB", [P, totB // 16], I16, isOutput=False)
    sblk = nc.declare_dram_parameter("sblk", [P, nblk * P], BF16, isOutput=False)
    xposT = nc.declare_dram_parameter("xposT", [2, padsh], F32, isOutput=False)
    xpos_nm = nc.declare_dram_parameter("xpos_nm", [padsh, 2], F32, isOutput=False)
    coarseT = nc.declare_dram_parameter("coarseT", [2, n_coarse], F32, isOutput=False)
    ycoarse = nc.declare_dram_parameter("ycoarse", [n_coarse, out_dim], F32, isOutput=False)
    w_mid = [nc.declare_dram_parameter(n, [hid, hid], F32, isOutput=False)
             for n in ("w1", "w2", "we0", "we1")]
    b_mid = [nc.declare_dram_parameter(n, [hid], F32, isOutput=False)
             for n in ("b1", "b2", "be0", "be1")]
    w0 = nc.declare_dram_parameter("w0", [6, hid], F32, isOutput=False)
    b0 = nc.declare_dram_parameter("b0", [hid], F32, isOutput=False)
    wtop = nc.declare_dram_parameter("wtop", [out_dim, hid], F32, isOutput=False)
    w5 = nc.declare_dram_parameter("w5", [hid, out_dim], F32, isOutput=False)
    b5 = nc.declare_dram_parameter("b5", [out_dim], F32, isOutput=False)
    y_out = nc.declare_dram_parameter("out", [padsh, out_dim], F32, isOutput=True)

    # ---- internal DRAM ----
    zshA = [nc.dram_tensor(f"zshA{i}", [CHA, hid], BF16) for i in range(4)]
    zshB = [nc.dram_tensor(f"zshB{i}", [CHB, hid], BF16) for i in range(4)]
    zfullA = [nc.dram_tensor(f"zfullA{i}", [rowsA, hid], BF16, addr_space="Shared")
              for i in range(4)]
    zfullB = [nc.dram_tensor(f"zfullB{i}", [rowsB, hid], BF16, addr_space="Shared")
              for i in range(4)]
    z5shA = nc.dram_tensor("z5shA", [CHA, P], BF16)
    z5shB = nc.dram_tensor("z5shB", [CHB, P], BF16)
    z5fullA = nc.dram_tensor("z5fullA", [rowsA, P], BF16, addr_space="Shared")
    z5fullB = nc.dram_tensor("z5fullB", [rowsB, P], BF16, addr_space="Shared")

    with TileContext(nc) as tc:
        with ExitStack() as ctx:
            main = ctx.enter_context(tc.tile_pool(name="main", bufs=1))
            sp = ctx.enter_context(tc.tile_pool(name="sp", bufs=3))
            mp = ctx.enter_context(tc.tile_pool(name="mp", bufs=2))
            smallp = ctx.enter_context(tc.tile_pool(name="smallp", bufs=2))
            ppA = ctx.enter_context(tc.tile_pool(name="ppA", bufs=2, space="PSUM"))
            ppB = ctx.enter_context(tc.tile_pool(name="ppB", bufs=2, space="PSUM"))
            ppC = ctx.enter_context(tc.tile_pool(name="ppC", bufs=2, space="PSUM"))

            def accps(shape):
                return ppA.tile(shape, F32, tag="acc", name="acc")

            def densps(shape):
                return ppB.tile(shape, F32, tag="dacc", name="dacc")

            def tps(shape):
                return ppC.tile(shape, F32, tag="tp", name="tp")

            # ---------- persistent tiles ----------
            hT = main.tile([P, kc, padsh], BF16, tag="hT")
            znm = main.tile([P, nt, hid], BF16, tag="znm")
            z5nm = main.tile([P, nt, 4], BF16, tag="z5nm")
            h0nm = main.tile([P, nt, 8], BF16, tag="h0nm")
            y3n = main.tile([P, nt, out_dim], F32, tag="y3n")
            iden = main.tile([P, P], F32, tag="iden")
            idxA_sb = main.tile([P, totA // 16], I16, tag="idxA_sb")
            idxB_sb = main.tile([P, totB // 16], I16, tag="idxB_sb")
            wtop_sb = main.tile([out_dim, hid], F32, tag="wtop_sb")
            w0_sb = main.tile([6, hid], F32, tag="w0_sb")
            w_sb = [main.tile([P, kc, hid], BF16, tag=f"w_sb{i}", name=f"w_sb{i}")
                    for i in range(4)]
            b_sb = [main.tile([P, kc], F32, tag=f"b_sb{i}", name=f"b_sb{i}")
                    for i in range(4)]
            b0_sb = main.tile([P, kc], F32, tag="b0_sb")
            w5_sb = main.tile([P, kc, out_dim], BF16, tag="w5_sb")
            b5_sb = main.tile([out_dim, 1], F32, tag="b5_sb")

            nc.sync.dma_start(out=idxA_sb[:], in_=idxA[:, :])
            nc.sync.dma_start(out=idxB_sb[:], in_=idxB[:, :])
            nc.sync.dma_start(out=wtop_sb[:], in_=wtop[:, :])
            nc.sync.dma_start(out=w0_sb[:], in_=w0[:, :])
            nc.sync.dma_start(
                out=h0nm[:], in_=h0nm_d[:, :].rearrange("(t p) d -> p t d", p=P))
            for i in range(4):
                nc.gpsimd.dma_start(
                    out=w_sb[i][:],
                    in_=w_mid[i][:, :].rearrange("(k p) h -> p k h", p=P))
                nc.sync.dma_start(
                    out=b_sb[i][:], in_=b_mid[i][:].rearrange("(k p) -> p k", p=P))
            nc.sync.dma_start(out=b0_sb[:], in_=b0[:].rearrange("(k p) -> p k", p=P))
            nc.gpsimd.dma_start(
                out=w5_sb[:], in_=w5[:, :].rearrange("(k p) o -> p k o", p=P))
            nc.sync.dma_start(out=b5_sb[:], in_=b5[:, None])
            make_identity(nc, iden[:])

            mm_pp = {
                "mmA": [main.tile([P, GA, hid], BF16, tag=f"mmA{i}",
                                  name=f"mmA{i}") for i in range(2)],
                "mmB": [main.tile([P, GB, hid], BF16, tag=f"mmB{i}",
                                  name=f"mmB{i}") for i in range(2)],
                "m6A": [main.tile([P, GA, P], BF16, tag=f"m6A{i}",
                                  name=f"m6A{i}") for i in range(2)],
                "m6B": [main.tile([P, GB, P], BF16, tag=f"m6B{i}",
                                  name=f"m6B{i}") for i in range(2)],
            }
            for pp in mm_pp.values():
                for m in pp:
                    nc.gpsimd.memset(m[:], 0.0)

            # ---------- helpers ----------
            def gather_group(tabA, tabB, g0, g1, elem, tag):
                nA = sum(SLA[t] for t in range(g0, g1))
                nB = sum(SLB[t] for t in range(g0, g1))
                nbA = math.ceil(nA / P)
                nbB = math.ceil(nB / P)
                gi = g0 // GRP
                mA = mm_pp[tag + "A"][gi % 2]
                mB = mm_pp[tag + "B"][gi % 2]
                nc.gpsimd.dma_gather(
                    mA[:, 0:nbA, :], tabA,
                    idxA_sb[:, sbaseA[g0] // 16: (sbaseA[g0] + nA) // 16],
                    nA, nA, elem)
                nc.gpsimd.dma_gather(
                    mB[:, 0:nbB, :], tabB,
                    idxB_sb[:, sbaseB[g0] // 16: (sbaseB[g0] + nB) // 16],
                    nB, nB, elem)
                return mA, mB

            def load_S(t):
                nbt = 1 + nsAv[t] + nsBv[t]
                St = sp.tile([P, nbt * P], BF16, tag="St", name="St")
                nc.sync.dma_start(
                    out=St[:], in_=sblk[:, colbase[t] * P:(colbase[t] + nbt) * P])
                return St

            def scatter_tile(t, g0, mA, mB, self_lhsT, acc, nrows, cchunks, St):
                for c0 in cchunks:
                    out = (acc[0:nrows, c0 * P:c0 * P + P] if nrows < P
                           else acc[:, c0 * P:c0 * P + P])
                    nc.tensor.matmul(out=out, lhsT=self_lhsT(c0),
                                     rhs=St[:, 0:P], start=True, stop=False)
                    for j, b in enumerate(range(bsA[t], beA[t])):
                        nc.tensor.matmul(
                            out=out,
                            lhsT=(mA[:, b, c0 * P:c0 * P + P] if nrows == P
                                  else mA[:, b, 0:nrows]),
                            rhs=St[:, (1 + j) * P:(2 + j) * P],
                            start=False, stop=False)
                    nsa = nsAv[t]
                    for j, b in enumerate(range(bsB[t], beB[t])):
                        nc.tensor.matmul(
                            out=out,
                            lhsT=(mB[:, b, c0 * P:c0 * P + P] if nrows == P
                                  else mB[:, b, 0:nrows]),
                            rhs=St[:, (1 + nsa + j) * P:(2 + nsa + j) * P],
                            start=False, stop=(j == nsBv[t] - 1))

            def dense_tile(li, t):
                # z_{li} = h @ W_li for tile t -> znm + zsh; AGs fired at
                # chunk boundaries so they overlap the remaining sparse work
                tp_ = t * P
                zps = densps([P, hid])
                for k in range(kc):
                    nc.tensor.matmul(out=zps[:], lhsT=hT[:, k, tp_:tp_ + P],
                                     rhs=w_sb[li][:, k, :], start=(k == 0),
                                     stop=(k == kc - 1) and li != 2)
                if li == 2:
                    pt3 = tps([P, P])
                    nc.tensor.transpose(out=pt3[0:out_dim, 0:P],
                                        in_=y3n[:, t, :], identity=iden[:])
                    y3t_T = smallp.tile([out_dim, P], F32, tag="y3t_T")
                    nc.vector.tensor_copy(out=y3t_T[:], in_=pt3[0:out_dim, 0:P])
                    nc.tensor.matmul(out=zps[:], lhsT=y3t_T[:],
                                     rhs=wtop_sb[:, :], start=False, stop=True)
                nc.scalar.activation(out=znm[:, t, :], in_=zps[:], func=AF.Copy)
                if t < TILES_A:
                    nc.sync.dma_start(out=zshA[li][tp_:tp_ + P, :], in_=znm[:, t, :])
                else:
                    nc.sync.dma_start(out=zshB[li][tp_ - CHA:tp_ - CHA + P, :],
                                      in_=znm[:, t, :])
                if t == TILES_A - 1:
                    nc.gpsimd.collective_compute(
                        "AllGather", ALU.bypass, replica_groups=rg,
                        ins=[zshA[li][:, :]], outs=[zfullA[li][:, :]])
                if t == nt - 1:
                    nc.gpsimd.collective_compute(
                        "AllGather", ALU.bypass, replica_groups=rg,
                        ins=[zshB[li][:, :]], outs=[zfullB[li][:, :]])

            def z5dense_tile(t):
                tp_ = t * P
                z5ps = densps([P, hid])
                for k in range(kc):
                    nc.tensor.matmul(out=z5ps[0:out_dim, 0:P], lhsT=w5_sb[:, k, :],
                                     rhs=hT[:, k, tp_:tp_ + P], start=(k == 0),
                                     stop=(k == kc - 1))
                z5T_sb = smallp.tile([out_dim, P], F32, tag="z5T_sb")
                nc.vector.tensor_copy(out=z5T_sb[:], in_=z5ps[0:out_dim, 0:P])
                ptp = tps([P, P])
                nc.tensor.transpose(out=ptp[:, 0:out_dim], in_=z5T_sb[:],
                                    identity=iden[0:out_dim, 0:out_dim])
                nc.vector.tensor_copy(out=z5nm[:, t, 0:out_dim],
                                      in_=ptp[:, 0:out_dim])
                if t < TILES_A:
                    nc.sync.dma_start(out=z5shA[tp_:tp_ + P, 0:out_dim],
                                      in_=z5nm[:, t, 0:out_dim])
                else:
                    nc.sync.dma_start(out=z5shB[tp_ - CHA:tp_ - CHA + P, 0:out_dim],
                                      in_=z5nm[:, t, 0:out_dim])
                if t == TILES_A - 1:
                    nc.gpsimd.collective_compute(
                        "AllGather", ALU.bypass, replica_groups=rg,
                        ins=[z5shA[:, :]], outs=[z5fullA[:, :]])
                if t == nt - 1:
                    nc.gpsimd.collective_compute(
                        "AllGather", ALU.bypass, replica_groups=rg,
                        ins=[z5shB[:, :]], outs=[z5fullB[:, :]])

            # ---------- pre0: q = A h0 (6-wide), z0 = W0^T q, relu; dense0 ----
            for g0, g1 in groups:
                mA, mB = gather_group(h0A[:, :], h0B[:, :], g0, g1, P, "m6")
                for t in range(g0, g1):
                    tp_ = t * P
                    St = load_S(t)
                    q = accps([P, P])
                    scatter_tile(t, g0, mA, mB, lambda c0: h0nm[:, t, 0:6],
                                 q, 6, [0], St)
                    q_sb = smallp.tile([6, P], F32, tag="q_sb")
                    nc.vector.tensor_copy(out=q_sb[:], in_=q[0:6, 0:P])
                    for jj in range(kc):
                        z0 = densps([P, P])
                        nc.tensor.matmul(out=z0[:, 0:P],
                                         lhsT=w0_sb[:, jj * P:(jj + 1) * P],
                                         rhs=q_sb[:], start=True, stop=True)
                        nc.scalar.activation(out=hT[:, jj, tp_:tp_ + P],
                                             in_=z0[:, 0:P], func=AF.Relu,
                                             bias=b0_sb[:, jj:jj + 1])
                    dense_tile(0, t)

            # ---------- KNN (independent; writes y3n) ----------
            with tc.tile_pool(name="knn", bufs=2) as kp:
                mones_sb = kp.tile([1, P], F32, tag="mones_sb", bufs=1)
                nc.vector.memset(mones_sb[:], -1.0)
                coarse3 = kp.tile([3, n_coarse], F32, tag="coarse3", bufs=1)
                with tc.tile_pool(name="knnprep", bufs=1) as kprep:
                    nc.sync.dma_start(out=coarse3[0:2, :], in_=coarseT[:, :])
                    pones = kprep.tile([2, 1], F32, tag="pones")
                    nc.vector.memset(pones[:], 1.0)
                    for i in range(ncc):
                        a, b = i * 512, min((i + 1) * 512, n_coarse)
                        sqc = kprep.tile([2, 512], F32, tag="sqc")
                        nc.vector.tensor_tensor(out=sqc[:, : b - a],
                                                in0=coarse3[0:2, a:b],
                                                in1=coarse3[0:2, a:b], op=ALU.mult)
                        ps = tps([P, 512])
                        nc.tensor.matmul(out=ps[0:1, : b - a], lhsT=pones[:],
                                         rhs=sqc[:, : b - a], start=True, stop=True)
                        csq = kprep.tile([1, 512], F32, tag="csq")
                        nc.vector.tensor_copy(out=csq[:, : b - a],
                                              in_=ps[0:1, : b - a])
                        nc.sync.dma_start(out=coarse3[2:3, a:b],
                                          in_=csq[:, : b - a])

                    xnm = kprep.tile([P, nt, 2], F32, tag="xnm")
                    nc.sync.dma_start(
                        out=xnm[:], in_=xpos_nm[:, :].rearrange("(t p) d -> p t d", p=P))
                    sqn = kprep.tile([P, nt, 2], F32, tag="sqn")
                    nc.vector.tensor_tensor(out=sqn[:], in0=xnm[:], in1=xnm[:],
                                            op=ALU.mult)
                    fsqneg = kp.tile([P, nt], F32, tag="fsqneg", bufs=1)
                    nc.vector.tensor_reduce(out=fsqneg[:], in_=sqn[:],
                                            axis=mybir.AxisListType.X, op=ALU.add,
                                            negate=True)

                for t in range(nt):
                    tp_ = t * P
                    xp_t = kp.tile([2, P], F32, tag="xp_t")
                    nc.sync.dma_start(out=xp_t[:], in_=xposT[:, tp_:tp_ + P])
                    lhsT3 = kp.tile([3, P], F32, tag="lhsT3")
                    nc.vector.tensor_scalar_mul(lhsT3[0:2, :], xp_t[:], 2.0)
                    nc.sync.dma_start(out=lhsT3[2:3, :], in_=mones_sb[:])

                    d2 = kp.tile([P, ncpad], F32, tag="d2", bufs=1)
                    for i in range(ncc):
                        a, b = i * 512, min((i + 1) * 512, n_coarse)
                        dps = densps([P, 512])
                        nc.tensor.matmul(out=dps[:, : b - a], lhsT=lhsT3[:],
                                         rhs=coarse3[:, a:b], start=True, stop=True)
                        nc.vector.tensor_scalar(out=d2[:, a:b], in0=dps[:, : b - a],
                                                scalar1=fsqneg[:, t:t + 1],
                                                scalar2=None, op0=ALU.add)
                    vals = kp.tile([P, 8], F32, tag="vals")
                    nc.vector.max(out=vals[:], in_=d2[:, 0:n_coarse])
                    idxs = kp.tile([P, 8], mybir.dt.uint32, tag="idxs")
                    nc.vector.max_index(out=idxs[:], in_max=vals[:],
                                        in_values=d2[:, 0:n_coarse])
                    wv = kp.tile([P, 3], F32, tag="wv")
                    nc.vector.tensor_scalar(out=wv[:], in0=vals[:, 0:3],
                                            scalar1=-1.0, scalar2=1e-16,
                                            op0=ALU.mult, op1=ALU.max)
                    nc.vector.reciprocal(out=wv[:], in_=wv[:])
                    wsum = kp.tile([P, 1], F32, tag="wsum")
                    nc.vector.tensor_reduce(out=wsum[:], in_=wv[:],
                                            axis=mybir.AxisListType.X, op=ALU.add)
                    nc.vector.reciprocal(out=wsum[:], in_=wsum[:])
                    nc.vector.tensor_scalar(out=wv[:], in0=wv[:],
                                            scalar1=wsum[:, 0:1], scalar2=None,
                                            op0=ALU.mult)
                    yg = kp.tile([P, 3, out_dim], F32, tag="yg")
                    for k3 in range(3):
                        nc.gpsimd.indirect_dma_start(
                            out=yg[:, k3, :], out_offset=None, in_=ycoarse[:, :],
                            in_offset=IOO(ap=idxs[:, k3:k3 + 1], axis=0))
                    tmp = kp.tile([P, out_dim], F32, tag="tmp")
                    nc.vector.tensor_scalar(out=y3n[:, t, :], in0=yg[:, 0, :],
                                            scalar1=wv[:, 0:1], scalar2=None,
                                            op0=ALU.mult)
                    for k in (1, 2):
                        nc.vector.tensor_scalar(out=tmp[:], in0=yg[:, k, :],
                                                scalar1=wv[:, k:k + 1], scalar2=None,
                                                op0=ALU.mult)
                        nc.vector.tensor_tensor(out=y3n[:, t, :], in0=y3n[:, t, :],
                                                in1=tmp[:], op=ALU.add)

            # ---------- mid layers: sparse(li) + interleaved dense(li+1) ----
            for li in range(4):
                for g0, g1 in groups:
                    mA, mB = gather_group(zfullA[li][:, :], zfullB[li][:, :],
                                          g0, g1, hid, "mm")
                    for t in range(g0, g1):
                        tp_ = t * P
                        St = load_S(t)
                        hps = accps([P, kc * P])
                        scatter_tile(t, g0, mA, mB,
                                     lambda c0: znm[:, t, c0 * P:c0 * P + P],
                                     hps, P, list(range(kc)), St)
                        for cc in range(kc):
                            nc.scalar.activation(out=hT[:, cc, tp_:tp_ + P],
                                                 in_=hps[:, cc * P:(cc + 1) * P],
                                                 func=AF.Relu,
                                                 bias=b_sb[li][:, cc:cc + 1])
                        if li < 3:
                            dense_tile(li + 1, t)
                        else:
                            z5dense_tile(t)

            # ---------- end2 final: out = A z5 + b5 ----------
            for g0, g1 in groups:
                mA, mB = gather_group(z5fullA[:, :], z5fullB[:, :], g0, g1, P, "m6")
                for t in range(g0, g1):
                    tp_ = t * P
                    St = load_S(t)
                    ops = accps([P, P])
                    scatter_tile(t, g0, mA, mB, lambda c0: z5nm[:, t, 0:out_dim],
                                 ops, out_dim, [0], St)
                    oT = smallp.tile([out_dim, P], F32, tag="oT")
                    nc.vector.tensor_scalar(out=oT[:], in0=ops[0:out_dim, 0:P],
                                            scalar1=b5_sb[:, 0:1], scalar2=None,
                                            op0=ALU.add)
                    po = tps([P, P])
                    nc.tensor.transpose(out=po[:, 0:out_dim], in_=oT[:],
                                        identity=iden[0:out_dim, 0:out_dim])
                    o_sb = smallp.tile([P, out_dim], F32, tag="o_sb")
                    nc.vector.tensor_copy(out=o_sb[:], in_=po[:, 0:out_dim])
                    nc.sync.dma_start(out=y_out[tp_:tp_ + P, :], in_=o_sb[:])

    nc.finalize()
    return nc


# ---------------------------------------------------------------- entry point
def _prepare(inputs, n_fine, n_coarse, hid, out_dim, ncores):
    import ml_dtypes
    bf16 = ml_dtypes.bfloat16
    x = np.asarray(inputs["x"], np.float32)
    sdf = np.asarray(inputs["sdf"], np.float32)
    coarse_x = np.asarray(inputs["coarse_x"], np.float32)
    coarse_y = np.asarray(inputs["coarse_y"], np.float32)
    edge_index = np.asarray(inputs["edge_index"])

    SLA, SLB, nt, padsh, edges = _preprocess_edges(edge_index, n_fine, ncores)
    nsh = n_fine // ncores

    h0 = np.zeros((n_fine, P), np.float32)
    h0[:, 0:5] = x
    h0[:, 5:6] = sdf
    h0A = np.zeros((ncores * CHA, P), bf16)
    h0B = np.zeros((ncores * CHB, P), bf16)
    for c in range(ncores):
        sh = h0[c * nsh:(c + 1) * nsh]
        h0A[c * CHA:(c + 1) * CHA] = sh[:CHA].astype(bf16)
        h0B[c * CHB:c * CHB + (nsh - CHA)] = sh[CHA:].astype(bf16)

    xpos = x[:, :2].astype(np.float32)
    coarseT = np.ascontiguousarray(coarse_x[:, :2].T).astype(np.float32)

    in_maps = []
    for c in range(ncores):
        xx = np.zeros((2, padsh), np.float32)
        xx[:, :nsh] = xpos[c * nsh:(c + 1) * nsh].T
        xn = np.zeros((padsh, 2), np.float32)
        xn[:nsh] = xpos[c * nsh:(c + 1) * nsh]
        h0nm = np.zeros((padsh, 8), bf16)
        h0nm[:nsh, 0:6] = h0[c * nsh:(c + 1) * nsh, 0:6].astype(bf16)
        m = {
            "h0A": h0A, "h0B": h0B, "h0nm": h0nm,
            "idxA": edges[c]["idxA"], "idxB": edges[c]["idxB"],
            "sblk": edges[c]["sblk"],
            "xposT": xx, "xpos_nm": xn,
            "coarseT": coarseT, "ycoarse": coarse_y,
            "w0": np.asarray(inputs["pre_W0"], np.float32),
            "b0": np.asarray(inputs["pre_b0"], np.float32),
            "w1": np.asarray(inputs["pre_W1"], np.float32),
            "b1": np.asarray(inputs["pre_b1"], np.float32),
            "w2": np.asarray(inputs["pre_W2"], np.float32),
            "b2": np.asarray(inputs["pre_b2"], np.float32),
            "wtop": np.ascontiguousarray(np.asarray(inputs["end_W0"], np.float32)[:out_dim]),
            "we0": np.ascontiguousarray(np.asarray(inputs["end_W0"], np.float32)[out_dim:]),
            "be0": np.asarray(inputs["end_b0"], np.float32),
            "we1": np.asarray(inputs["end_W1"], np.float32),
            "be1": np.asarray(inputs["end_b1"], np.float32),
            "w5": np.asarray(inputs["end_W2"], np.float32),
            "b5": np.asarray(inputs["end_b2"], np.float32),
        }
        in_maps.append(m)
    return SLA, SLB, nt, padsh, in_maps


def run(inputs, n_fine=N_FINE, n_coarse=N_COARSE, hid=HID, out_dim=OUT,
        ncores=NCORES, sim=False, trace=False):
    SLA, SLB, nt, padsh, in_maps = _prepare(inputs, n_fine, n_coarse, hid,
                                            out_dim, ncores)
    key = (n_fine, n_coarse, hid, out_dim, ncores, tuple(SLA), tuple(SLB), nt)
    if key not in _PROGRAM_CACHE:
        _PROGRAM_CACHE[key] = build_program(n_fine, n_coarse, hid, out_dim,
                                            ncores, SLA, SLB, nt)
    nc = _PROGRAM_CACHE[key]

    nsh = n_fine // ncores
    if sim:
        from concourse.bass_interp import MultiCoreSim
        ms = MultiCoreSim(nc, ncores, num_workers=1)
        for c in range(ncores):
            for k, v in in_maps[c].items():
                ms.cores[c].tensor(k)[:] = v
        ms.simulate()
        outs = [np.array(ms.cores[c].tensor("out")) for c in range(ncores)]
        exec_ns = None
    else:
        from concourse.bass_utils import run_bass_kernel_spmd
        res = run_bass_kernel_spmd(nc, in_maps, list(range(ncores)), trace=trace)
        outs = [res.results[c]["out"] for c in range(ncores)]
        exec_ns = res.exec_time_ns

    full = np.zeros((n_fine, out_dim), np.float32)
    for c in range(ncores):
        full[c * nsh:(c + 1) * nsh] = outs[c][:nsh]
    return full, exec_ns


def kernel(**inputs):
    out, _ = run(inputs)
    return out


# revision 18
# speedup vs baseline: 1.1635x; 1.0170x over previous
"""CFD-GCN Trainium2 kernel: 6-layer GCN + KNN-interpolate on 8 NeuronCores.

v3 strategy (node sharding, feature-major residency, bf16 sparse path):
  - Fine nodes sharded 6250/core (padded 6272 = 49*128 = nt tiles).
  - Per GCN layer: z = h @ W (dense, bf16, PE) kept node-major in SBUF
    (znm) AND written to DRAM shards zshA/zshB split by local row range
    (A = rows 0:3200 / tiles 0-24, B = rows 3200:6272 / tiles 25-48).
    The dense matmuls for layer l+1 are interleaved per-tile into the
    sparse phase of layer l, so AllGather-A fires ~halfway through the
    sparse phase and AllGather-B at its end -- both mostly hidden.
    Tables zfullA [8*3200, 512] / zfullB [8*3072, 512] keep row spaces
    within int16 gather-index range.
  - Edge gather: one dma_gather per (chunk, group of 2 dest tiles) with
    compile-time num_idxs (per-tile slots = max-over-cores count padded
    to x128 with dummy idx 0), no count registers.
  - Scatter-add: one-hot S blocks precomputed on the HOST as dense bf16
    [128 x 128] blocks (layout per tile [selfloop | A | B]) streamed
    from DRAM -- no on-device S construction. Self-loops are not in the
    edge lists: the diagonal selfloop block multiplies the SBUF-resident
    node-major z (dinv^2 * z) at zero gather cost.
  - pre0 (A h0 then W0) and end2 (W5 then A z5) run the same sparse
    machinery in bf16 against 128-wide tables h0A/h0B, z5fullA/z5fullB,
    reusing the same index tables and S blocks.
  - KNN-interpolate: matmul d2, DVE max8/max_index, small indirect
    gathers of coarse_y; overlaps the pre0 phase.
"""

import math
import numpy as np

# ---------------------------------------------------------------- constants
N_FINE = 50000
N_COARSE = 2000
HID = 512
OUT = 3
NCORES = 8
P = 128
CHA = 3200          # chunk A local rows (tiles 0..24)
CHB = 3072          # chunk B local rows (tiles 25..48)
TILES_A = CHA // P  # 25
GRP = 2             # dest tiles per gather group

_PROGRAM_CACHE = {}


# ---------------------------------------------------------------- host side
def _wrap16(flat, P=128):
    L = len(flat) // 16
    w = np.asarray(flat, np.int16).reshape(L, 16).T  # [16, L]
    return np.tile(w, (P // 16, 1))


def _mk_groups(nt):
    groups = []
    t0 = 0
    while t0 < nt:
        groups.append((t0, min(t0 + GRP, nt)))
        t0 = min(t0 + GRP, nt)
    return groups


def _slot_layout(SL, nt):
    """Per-chunk layout: group-relative offsets, block spans, flat base."""
    groups = _mk_groups(nt)
    rel = [0] * nt
    slotbase = {}
    base = 0
    for g0, g1 in groups:
        r = 0
        for t in range(g0, g1):
            rel[t] = r
            r += SL[t]
        slotbase[g0] = base
        base += r
    bs = [rel[t] // P for t in range(nt)]
    be = [math.ceil((rel[t] + SL[t]) / P) for t in range(nt)]
    return groups, rel, slotbase, base, bs, be


def _preprocess_edges(edge_index, n_fine, ncores):
    """Dest-sorted edge lists split by source chunk + host-built S blocks.

    Slots are packed at 16 granularity (per tile: max-over-cores count
    rounded to x16); matmul blocks follow the group-relative 128 grid, so
    a block shared by two tiles gets two complementary masked S columns.
    """
    import ml_dtypes
    bf16 = ml_dtypes.bfloat16
    nsh = n_fine // ncores              # 6250
    nt = math.ceil(nsh / P)             # 49
    padsh = nt * P                      # 6272

    row = np.asarray(edge_index[0]).astype(np.int64)
    col = np.asarray(edge_index[1]).astype(np.int64)

    deg = (np.bincount(col, minlength=n_fine) + 1.0).astype(np.float32)
    dinv = 1.0 / np.sqrt(deg)
    normv = (dinv[row] * dinv[col]).astype(np.float32)
    dinv2 = (dinv * dinv).astype(np.float32)

    order = np.argsort(col, kind="stable")
    col_s, row_s, norm_s = col[order], row[order], normv[order]

    src_core = row_s // nsh
    src_ls = row_s % nsh
    isa = src_ls < CHA
    idxA_val = src_core * CHA + src_ls
    idxB_val = src_core * CHB + (src_ls - CHA)

    cnt = np.zeros((ncores, nt, 2), np.int64)
    bounds = {}
    for c in range(ncores):
        base = c * nsh
        for t in range(nt):
            lo, hi = base + t * P, min(base + (t + 1) * P, base + nsh)
            a = np.searchsorted(col_s, lo, "left")
            b = np.searchsorted(col_s, hi, "left")
            na = int(isa[a:b].sum())
            cnt[c, t, 0] = na
            cnt[c, t, 1] = (b - a) - na
            bounds[(c, t)] = (a, b)

    SLA = [16 * int(math.ceil(max(1, cnt[:, t, 0].max()) / 16)) for t in range(nt)]
    SLB = [16 * int(math.ceil(max(1, cnt[:, t, 1].max()) / 16)) for t in range(nt)]
    groups, relA, sbaseA, totA, bsA, beA = _slot_layout(SLA, nt)
    _, relB, sbaseB, totB, bsB, beB = _slot_layout(SLB, nt)
    g_of = {}
    for g0, g1 in groups:
        for t in range(g0, g1):
            g_of[t] = g0
    nsAv = [beA[t] - bsA[t] for t in range(nt)]
    nsBv = [beB[t] - bsB[t] for t in range(nt)]
    colbase = [0] * nt
    acc = 0
    for t in range(nt):
        colbase[t] = acc
        acc += 1 + nsAv[t] + nsBv[t]
    nblk = acc

    dvec = np.arange(P, dtype=np.float32)
    out = []
    for c in range(ncores):
        flatA = np.zeros(totA, np.int64)
        flatB = np.zeros(totB, np.int64)
        ecol = np.full((P, nblk), -1.0, np.float32)
        enorm = np.zeros((P, nblk), np.float32)
        base = c * nsh
        for t in range(nt):
            a, b = bounds[(c, t)]
            m = isa[a:b]
            crel = (col_s[a:b] - (base + t * P)).astype(np.float32)
            nrm = norm_s[a:b]
            cb = colbase[t]
            nvalid = min(nsh - t * P, P)
            pp = np.arange(nvalid)
            ecol[pp, cb] = pp
            enorm[pp, cb] = dinv2[base + t * P: base + t * P + nvalid]
            for ids, rel, sbase, bs, flat, bcol in (
                    (idxA_val[a:b][m], relA[t], sbaseA[g_of[t]], bsA[t],
                     flatA, cb + 1),
                    (idxB_val[a:b][~m], relB[t], sbaseB[g_of[t]], bsB[t],
                     flatB, cb + 1 + nsAv[t])):
                n = len(ids)
                flat[sbase + rel: sbase + rel + n] = ids
                q = rel + np.arange(n)
                cc = crel[m] if flat is flatA else crel[~m]
                nn = nrm[m] if flat is flatA else nrm[~m]
                ecol[q % P, bcol + q // P - bs] = cc
                enorm[q % P, bcol + q // P - bs] = nn
        sblk = ((ecol[:, :, None] == dvec[None, None, :])
                * enorm[:, :, None]).astype(bf16).reshape(P, nblk * P)
        out.append({
            "idxA": _wrap16(flatA), "idxB": _wrap16(flatB),
            "sblk": sblk,
        })
    return SLA, SLB, nt, padsh, out


# ---------------------------------------------------------------- device side
def build_program(n_fine, n_coarse, hid, out_dim, ncores, SLA, SLB, nt):
    import concourse.bass as bass
    import concourse.mybir as mybir
    from concourse.bacc import Bacc
    from concourse.tile import TileContext
    from concourse.masks import make_identity
    from contextlib import ExitStack

    F32 = mybir.dt.float32
    BF16 = mybir.dt.bfloat16
    I16 = mybir.dt.int16
    padsh = nt * P
    kc = hid // P
    rg = [list(range(ncores))]
    AF = mybir.ActivationFunctionType
    ALU = mybir.AluOpType
    IOO = bass.IndirectOffsetOnAxis
    ncpad = math.ceil(n_coarse / 512) * 512
    ncc = math.ceil(n_coarse / 512)

    groups, relA, sbaseA, totA, bsA, beA = _slot_layout(SLA, nt)
    _, relB, sbaseB, totB, bsB, beB = _slot_layout(SLB, nt)
    nsAv = [beA[t] - bsA[t] for t in range(nt)]
    nsBv = [beB[t] - bsB[t] for t in range(nt)]
    colbase = [0] * nt
    _acc = 0
    for t in range(nt):
        colbase[t] = _acc
        _acc += 1 + nsAv[t] + nsBv[t]
    nblk = _acc
    rowsA, rowsB = ncores * CHA, ncores * CHB
    GA = max(math.ceil(sum(SLA[t] for t in range(g0, g1)) / P)
             for g0, g1 in groups)
    GB = max(math.ceil(sum(SLB[t] for t in range(g0, g1)) / P)
             for g0, g1 in groups)

    nc = Bacc(num_devices=ncores)

    # ---- kernel I/O (per core) ----
    h0A = nc.declare_dram_parameter("h0A", [rowsA, P], BF16, isOutput=False)
    h0B = nc.declare_dram_parameter("h0B", [rowsB, P], BF16, isOutput=False)
    h0nm_d = nc.declare_dram_parameter("h0nm", [padsh, 8], BF16, isOutput=False)
    idxA = nc.declare_dram_parameter("idxA", [P, totA // 16], I16, isOutput=False)
    idxB = nc.declare_dram_parameter("idxB", [P, totB // 16], I16, isOutput=False)
    sblk = nc.declare_dram_parameter("sblk", [P, nblk * P], BF16, isOutput=False)
    xposT = nc.declare_dram_parameter("xposT", [2, padsh], F32, isOutput=False)
    xpos_nm = nc.declare_dram_parameter("xpos_nm", [padsh, 2], F32, isOutput=False)
    coarseT = nc.declare_dram_parameter("coarseT", [2, n_coarse], F32, isOutput=False)
    ycoarse = nc.declare_dram_parameter("ycoarse", [n_coarse, out_dim], F32, isOutput=False)
    w_mid = [nc.declare_dram_parameter(n, [hid, hid], F32, isOutput=False)
             for n in ("w1", "w2", "we0", "we1")]
    b_mid = [nc.declare_dram_parameter(n, [hid], F32, isOutput=False)
             for n in ("b1", "b2", "be0", "be1")]
    w0 = nc.declare_dram_parameter("w0", [6, hid], F32, isOutput=False)
    b0 = nc.declare_dram_parameter("b0", [hid], F32, isOutput=False)
    wtop = nc.declare_dram_parameter("wtop", [out_dim, hid], F32, isOutput=False)
    w5 = nc.declare_dram_parameter("w5", [hid, out_dim], F32, isOutput=False)
    b5 = nc.declare_dram_parameter("b5", [out_dim], F32, isOutput=False)
    y_out = nc.declare_dram_parameter("out", [padsh, out_dim], F32, isOutput=True)

    # ---- internal DRAM ----
    zshA = [nc.dram_tensor(f"zshA{i}", [CHA, hid], BF16) for i in range(4)]
    zshB = [nc.dram_tensor(f"zshB{i}", [CHB, hid], BF16) for i in range(4)]
    zfullA = [nc.dram_tensor(f"zfullA{i}", [rowsA, hid], BF16, addr_space="Shared")
              for i in range(4)]
    zfullB = [nc.dram_tensor(f"zfullB{i}", [rowsB, hid], BF16, addr_space="Shared")
              for i in range(4)]
    z5shA = nc.dram_tensor("z5shA", [CHA, P], BF16)
    z5shB = nc.dram_tensor("z5shB", [CHB, P], BF16)
    z5fullA = nc.dram_tensor("z5fullA", [rowsA, P], BF16, addr_space="Shared")
    z5fullB = nc.dram_tensor("z5fullB", [rowsB, P], BF16, addr_space="Shared")

    with TileContext(nc) as tc:
        with ExitStack() as ctx:
            main = ctx.enter_context(tc.tile_pool(name="main", bufs=1))
            sp = ctx.enter_context(tc.tile_pool(name="sp", bufs=3))
            mp = ctx.enter_context(tc.tile_pool(name="mp", bufs=2))
            smallp = ctx.enter_context(tc.tile_pool(name="smallp", bufs=2))
            ppA = ctx.enter_context(tc.tile_pool(name="ppA", bufs=2, space="PSUM"))
            ppB = ctx.enter_context(tc.tile_pool(name="ppB", bufs=2, space="PSUM"))
            ppC = ctx.enter_context(tc.tile_pool(name="ppC", bufs=2, space="PSUM"))

            def accps(shape):
                return ppA.tile(shape, F32, tag="acc", name="acc")

            def densps(shape):
                return ppB.tile(shape, F32, tag="dacc", name="dacc")

            def tps(shape):
                return ppC.tile(shape, F32, tag="tp", name="tp")

            # ---------- persistent tiles ----------
            hT = main.tile([P, kc, padsh], BF16, tag="hT")
            znm = main.tile([P, nt, hid], BF16, tag="znm")
            z5nm = main.tile([P, nt, 4], BF16, tag="z5nm")
            h0nm = main.tile([P, nt, 8], BF16, tag="h0nm")
            y3n = main.tile([P, nt, out_dim], F32, tag="y3n")
            iden = main.tile([P, P], F32, tag="iden")
            idxA_sb = main.tile([P, totA // 16], I16, tag="idxA_sb")
            idxB_sb = main.tile([P, totB // 16], I16, tag="idxB_sb")
            wtop_sb = main.tile([out_dim, hid], F32, tag="wtop_sb")
            w0_sb = main.tile([6, hid], F32, tag="w0_sb")
            w_sb = [main.tile([P, kc, hid], BF16, tag=f"w_sb{i}", name=f"w_sb{i}")
                    for i in range(4)]
            b_sb = [main.tile([P, kc], F32, tag=f"b_sb{i}", name=f"b_sb{i}")
                    for i in range(4)]
            b0_sb = main.tile([P, kc], F32, tag="b0_sb")
            w5_sb = main.tile([P, kc, out_dim], BF16, tag="w5_sb")
            b5_sb = main.tile([out_dim, 1], F32, tag="b5_sb")

            nc.sync.dma_start(out=idxA_sb[:], in_=idxA[:, :])
            nc.sync.dma_start(out=idxB_sb[:], in_=idxB[:, :])
            nc.sync.dma_start(out=wtop_sb[:], in_=wtop[:, :])
            nc.sync.dma_start(out=w0_sb[:], in_=w0[:, :])
            nc.sync.dma_start(
                out=h0nm[:], in_=h0nm_d[:, :].rearrange("(t p) d -> p t d", p=P))
            for i in range(4):
                nc.gpsimd.dma_start(
                    out=w_sb[i][:],
                    in_=w_mid[i][:, :].rearrange("(k p) h -> p k h", p=P))
                nc.sync.dma_start(
                    out=b_sb[i][:], in_=b_mid[i][:].rearrange("(k p) -> p k", p=P))
            nc.sync.dma_start(out=b0_sb[:], in_=b0[:].rearrange("(k p) -> p k", p=P))
            nc.gpsimd.dma_start(
                out=w5_sb[:], in_=w5[:, :].rearrange("(k p) o -> p k o", p=P))
            nc.sync.dma_start(out=b5_sb[:], in_=b5[:, None])
            make_identity(nc, iden[:])

            mm_pp = {
                "mmA": [main.tile([P, GA, hid], BF16, tag=f"mmA{i}",
                                  name=f"mmA{i}") for i in range(2)],
                "mmB": [main.tile([P, GB, hid], BF16, tag=f"mmB{i}",
                                  name=f"mmB{i}") for i in range(2)],
                "m6A": [main.tile([P, GA, P], BF16, tag=f"m6A{i}",
                                  name=f"m6A{i}") for i in range(3)],
                "m6B": [main.tile([P, GB, P], BF16, tag=f"m6B{i}",
                                  name=f"m6B{i}") for i in range(3)],
            }
            for pp in mm_pp.values():
                for m in pp:
                    nc.gpsimd.memset(m[:], 0.0)

            # ---------- helpers ----------
            def gather_group(tabA, tabB, g0, g1, elem, tag):
                nA = sum(SLA[t] for t in range(g0, g1))
                nB = sum(SLB[t] for t in range(g0, g1))
                nbA = math.ceil(nA / P)
                nbB = math.ceil(nB / P)
                gi = g0 // GRP
                mA = mm_pp[tag + "A"][gi % len(mm_pp[tag + "A"])]
                mB = mm_pp[tag + "B"][gi % len(mm_pp[tag + "B"])]
                nc.gpsimd.dma_gather(
                    mA[:, 0:nbA, :], tabA,
                    idxA_sb[:, sbaseA[g0] // 16: (sbaseA[g0] + nA) // 16],
                    nA, nA, elem)
                nc.gpsimd.dma_gather(
                    mB[:, 0:nbB, :], tabB,
                    idxB_sb[:, sbaseB[g0] // 16: (sbaseB[g0] + nB) // 16],
                    nB, nB, elem)
                return mA, mB

            def load_S(t):
                nbt = 1 + nsAv[t] + nsBv[t]
                St = sp.tile([P, nbt * P], BF16, tag="St", name="St")
                nc.scalar.dma_start(
                    out=St[:], in_=sblk[:, colbase[t] * P:(colbase[t] + nbt) * P])
                return St

            def scatter_tile(t, g0, mA, mB, self_lhsT, acc, nrows, cchunks, St):
                for c0 in cchunks:
                    out = (acc[0:nrows, c0 * P:c0 * P + P] if nrows < P
                           else acc[:, c0 * P:c0 * P + P])
                    nc.tensor.matmul(out=out, lhsT=self_lhsT(c0),
                                     rhs=St[:, 0:P], start=True, stop=False)
                    for j, b in enumerate(range(bsA[t], beA[t])):
                        nc.tensor.matmul(
                            out=out,
                            lhsT=(mA[:, b, c0 * P:c0 * P + P] if nrows == P
                                  else mA[:, b, 0:nrows]),
                            rhs=St[:, (1 + j) * P:(2 + j) * P],
                            start=False, stop=False)
                    nsa = nsAv[t]
                    for j, b in enumerate(range(bsB[t], beB[t])):
                        nc.tensor.matmul(
                            out=out,
                            lhsT=(mB[:, b, c0 * P:c0 * P + P] if nrows == P
                                  else mB[:, b, 0:nrows]),
                            rhs=St[:, (1 + nsa + j) * P:(2 + nsa + j) * P],
                            start=False, stop=(j == nsBv[t] - 1))

            def dense_tile(li, t):
                # z_{li} = h @ W_li for tile t -> znm + zsh; AGs fired at
                # chunk boundaries so they overlap the remaining sparse work
                tp_ = t * P
                zps = densps([P, hid])
                for k in range(kc):
                    nc.tensor.matmul(out=zps[:], lhsT=hT[:, k, tp_:tp_ + P],
                                     rhs=w_sb[li][:, k, :], start=(k == 0),
                                     stop=(k == kc - 1) and li != 2)
                if li == 2:
                    pt3 = tps([P, P])
                    nc.tensor.transpose(out=pt3[0:out_dim, 0:P],
                                        in_=y3n[:, t, :], identity=iden[:])
                    y3t_T = smallp.tile([out_dim, P], F32, tag="y3t_T")
                    nc.vector.tensor_copy(out=y3t_T[:], in_=pt3[0:out_dim, 0:P])
                    nc.tensor.matmul(out=zps[:], lhsT=y3t_T[:],
                                     rhs=wtop_sb[:, :], start=False, stop=True)
                nc.scalar.activation(out=znm[:, t, :], in_=zps[:], func=AF.Copy)
                if t < TILES_A:
                    nc.sync.dma_start(out=zshA[li][tp_:tp_ + P, :], in_=znm[:, t, :])
                else:
                    nc.sync.dma_start(out=zshB[li][tp_ - CHA:tp_ - CHA + P, :],
                                      in_=znm[:, t, :])
                if t == TILES_A - 1:
                    nc.gpsimd.collective_compute(
                        "AllGather", ALU.bypass, replica_groups=rg,
                        ins=[zshA[li][:, :]], outs=[zfullA[li][:, :]])
                if t == nt - 1:
                    nc.gpsimd.collective_compute(
                        "AllGather", ALU.bypass, replica_groups=rg,
                        ins=[zshB[li][:, :]], outs=[zfullB[li][:, :]])

            def z5dense_tile(t):
                tp_ = t * P
                z5ps = densps([P, hid])
                for k in range(kc):
                    nc.tensor.matmul(out=z5ps[0:out_dim, 0:P], lhsT=w5_sb[:, k, :],
                                     rhs=hT[:, k, tp_:tp_ + P], start=(k == 0),
                                     stop=(k == kc - 1))
                z5T_sb = smallp.tile([out_dim, P], F32, tag="z5T_sb")
                nc.vector.tensor_copy(out=z5T_sb[:], in_=z5ps[0:out_dim, 0:P])
                ptp = tps([P, P])
                nc.tensor.transpose(out=ptp[:, 0:out_dim], in_=z5T_sb[:],
                                    identity=iden[0:out_dim, 0:out_dim])
                nc.vector.tensor_copy(out=z5nm[:, t, 0:out_dim],
                                      in_=ptp[:, 0:out_dim])
                if t < TILES_A:
                    nc.sync.dma_start(out=z5shA[tp_:tp_ + P, 0:out_dim],
                                      in_=z5nm[:, t, 0:out_dim])
                else:
                    nc.sync.dma_start(out=z5shB[tp_ - CHA:tp_ - CHA + P, 0:out_dim],
                                      in_=z5nm[:, t, 0:out_dim])
                if t == TILES_A - 1:
                    nc.gpsimd.collective_compute(
                        "AllGather", ALU.bypass, replica_groups=rg,
                        ins=[z5shA[:, :]], outs=[z5fullA[:, :]])
                if t == nt - 1:
                    nc.gpsimd.collective_compute(
                        "AllGather", ALU.bypass, replica_groups=rg,
                        ins=[z5shB[:, :]], outs=[z5fullB[:, :]])

            # ---------- pre0: q = A h0 (6-wide), z0 = W0^T q, relu; dense0 ----
            for g0, g1 in groups:
                mA, mB = gather_group(h0A[:, :], h0B[:, :], g0, g1, P, "m6")
                for t in range(g0, g1):
                    tp_ = t * P
                    St = load_S(t)
                    q = accps([P, P])
                    scatter_tile(t, g0, mA, mB, lambda c0: h0nm[:, t, 0:6],
                                 q, 6, [0], St)
                    q_sb = smallp.tile([6, P], F32, tag="q_sb")
                    nc.vector.tensor_copy(out=q_sb[:], in_=q[0:6, 0:P])
                    for jj in range(kc):
                        z0 = densps([P, P])
                        nc.tensor.matmul(out=z0[:, 0:P],
                                         lhsT=w0_sb[:, jj * P:(jj + 1) * P],
                                         rhs=q_sb[:], start=True, stop=True)
                        nc.scalar.activation(out=hT[:, jj, tp_:tp_ + P],
                                             in_=z0[:, 0:P], func=AF.Relu,
                                             bias=b0_sb[:, jj:jj + 1])
                    dense_tile(0, t)

            # ---------- KNN (independent; writes y3n) ----------
            with tc.tile_pool(name="knn", bufs=2) as kp:
                mones_sb = kp.tile([1, P], F32, tag="mones_sb", bufs=1)
                nc.vector.memset(mones_sb[:], -1.0)
                coarse3 = kp.tile([3, n_coarse], F32, tag="coarse3", bufs=1)
                with tc.tile_pool(name="knnprep", bufs=1) as kprep:
                    nc.sync.dma_start(out=coarse3[0:2, :], in_=coarseT[:, :])
                    pones = kprep.tile([2, 1], F32, tag="pones")
                    nc.vector.memset(pones[:], 1.0)
                    for i in range(ncc):
                        a, b = i * 512, min((i + 1) * 512, n_coarse)
                        sqc = kprep.tile([2, 512], F32, tag="sqc")
                        nc.vector.tensor_tensor(out=sqc[:, : b - a],
                                                in0=coarse3[0:2, a:b],
                                                in1=coarse3[0:2, a:b], op=ALU.mult)
                        ps = tps([P, 512])
                        nc.tensor.matmul(out=ps[0:1, : b - a], lhsT=pones[:],
                                         rhs=sqc[:, : b - a], start=True, stop=True)
                        csq = kprep.tile([1, 512], F32, tag="csq")
                        nc.vector.tensor_copy(out=csq[:, : b - a],
                                              in_=ps[0:1, : b - a])
                        nc.sync.dma_start(out=coarse3[2:3, a:b],
                                          in_=csq[:, : b - a])

                    xnm = kprep.tile([P, nt, 2], F32, tag="xnm")
                    nc.sync.dma_start(
                        out=xnm[:], in_=xpos_nm[:, :].rearrange("(t p) d -> p t d", p=P))
                    sqn = kprep.tile([P, nt, 2], F32, tag="sqn")
                    nc.vector.tensor_tensor(out=sqn[:], in0=xnm[:], in1=xnm[:],
                                            op=ALU.mult)
                    fsqneg = kp.tile([P, nt], F32, tag="fsqneg", bufs=1)
                    nc.vector.tensor_reduce(out=fsqneg[:], in_=sqn[:],
                                            axis=mybir.AxisListType.X, op=ALU.add,
                                            negate=True)

                for t in range(nt):
                    tp_ = t * P
                    xp_t = kp.tile([2, P], F32, tag="xp_t")
                    nc.sync.dma_start(out=xp_t[:], in_=xposT[:, tp_:tp_ + P])
                    lhsT3 = kp.tile([3, P], F32, tag="lhsT3")
                    nc.vector.tensor_scalar_mul(lhsT3[0:2, :], xp_t[:], 2.0)
                    nc.sync.dma_start(out=lhsT3[2:3, :], in_=mones_sb[:])

                    d2 = kp.tile([P, ncpad], F32, tag="d2", bufs=1)
                    for i in range(ncc):
                        a, b = i * 512, min((i + 1) * 512, n_coarse)
                        dps = densps([P, 512])
                        nc.tensor.matmul(out=dps[:, : b - a], lhsT=lhsT3[:],
                                         rhs=coarse3[:, a:b], start=True, stop=True)
                        nc.vector.tensor_scalar(out=d2[:, a:b], in0=dps[:, : b - a],
                                                scalar1=fsqneg[:, t:t + 1],
                                                scalar2=None, op0=ALU.add)
                    vals = kp.tile([P, 8], F32, tag="vals")
                    nc.vector.max(out=vals[:], in_=d2[:, 0:n_coarse])
                    idxs = kp.tile([P, 8], mybir.dt.uint32, tag="idxs")
                    nc.vector.max_index(out=idxs[:], in_max=vals[:],
                                        in_values=d2[:, 0:n_coarse])
                    wv = kp.tile([P, 3], F32, tag="wv")
                    nc.vector.tensor_scalar(out=wv[:], in0=vals[:, 0:3],
                                            scalar1=-1.0, scalar2=1e-16,
                                            op0=ALU.mult, op1=ALU.max)
                    nc.vector.reciprocal(out=wv[:], in_=wv[:])
                    wsum = kp.tile([P, 1], F32, tag="wsum")
                    nc.vector.tensor_reduce(out=wsum[:], in_=wv[:],
                                            axis=mybir.AxisListType.X, op=ALU.add)
                    nc.vector.reciprocal(out=wsum[:], in_=wsum[:])
                    nc.vector.tensor_scalar(out=wv[:], in0=wv[:],
                                            scalar1=wsum[:, 0:1], scalar2=None,
                                            op0=ALU.mult)
                    yg = kp.tile([P, 3, out_dim], F32, tag="yg")
                    for k3 in range(3):
                        nc.gpsimd.indirect_dma_start(
                            out=yg[:, k3, :], out_offset=None, in_=ycoarse[:, :],
                            in_offset=IOO(ap=idxs[:, k3:k3 + 1], axis=0))
                    tmp = kp.tile([P, out_dim], F32, tag="tmp")
                    nc.vector.tensor_scalar(out=y3n[:, t, :], in0=yg[:, 0, :],
                                            scalar1=wv[:, 0:1], scalar2=None,
                                            op0=ALU.mult)
                    for k in (1, 2):
                        nc.vector.tensor_scalar(out=tmp[:], in0=yg[:, k, :],
                                                scalar1=wv[:, k:k + 1], scalar2=None,
                                                op0=ALU.mult)
                        nc.vector.tensor_tensor(out=y3n[:, t, :], in0=y3n[:, t, :],
                                                in1=tmp[:], op=ALU.add)

            # ---------- mid layers: sparse(li) + interleaved dense(li+1) ----
            for li in range(4):
                for g0, g1 in groups:
                    mA, mB = gather_group(zfullA[li][:, :], zfullB[li][:, :],
                                          g0, g1, hid, "mm")
                    for t in range(g0, g1):
                        tp_ = t * P
                        St = load_S(t)
                        hps = accps([P, kc * P])
                        scatter_tile(t, g0, mA, mB,
                                     lambda c0: znm[:, t, c0 * P:c0 * P + P],
                                     hps, P, list(range(kc)), St)
                        for cc in range(kc):
                            nc.scalar.activation(out=hT[:, cc, tp_:tp_ + P],
                                                 in_=hps[:, cc * P:(cc + 1) * P],
                                                 func=AF.Relu,
                                                 bias=b_sb[li][:, cc:cc + 1])
                        if li < 3:
                            dense_tile(li + 1, t)
                        else:
                            z5dense_tile(t)

            # ---------- end2 final: out = A z5 + b5 ----------
            for g0, g1 in groups:
                mA, mB = gather_group(z5fullA[:, :], z5fullB[:, :], g0, g1, P, "m6")
                for t in range(g0, g1):
                    tp_ = t * P
                    St = load_S(t)
                    ops = accps([P, P])
                    scatter_tile(t, g0, mA, mB, lambda c0: z5nm[:, t, 0:out_dim],
                                 ops, out_dim, [0], St)
                    oT = smallp.tile([out_dim, P], F32, tag="oT")
                    nc.vector.tensor_scalar(out=oT[:], in0=ops[0:out_dim, 0:P],
                                            scalar1=b5_sb[:, 0:1], scalar2=None,
                                            op0=ALU.add)
                    po = tps([P, P])
                    nc.tensor.transpose(out=po[:, 0:out_dim], in_=oT[:],
                                        identity=iden[0:out_dim, 0:out_dim])
                    o_sb = smallp.tile([P, out_dim], F32, tag="o_sb")
                    nc.vector.tensor_copy(out=o_sb[:], in_=po[:, 0:out_dim])
                    nc.sync.dma_start(out=y_out[tp_:tp_ + P, :], in_=o_sb[:])

    nc.finalize()
    return nc


# ---------------------------------------------------------------- entry point
def _prepare(inputs, n_fine, n_coarse, hid, out_dim, ncores):
    import ml_dtypes
    bf16 = ml_dtypes.bfloat16
    x = np.asarray(inputs["x"], np.float32)
    sdf = np.asarray(inputs["sdf"], np.float32)
    coarse_x = np.asarray(inputs["coarse_x"], np.float32)
    coarse_y = np.asarray(inputs["coarse_y"], np.float32)
    edge_index = np.asarray(inputs["edge_index"])

    SLA, SLB, nt, padsh, edges = _preprocess_edges(edge_index, n_fine, ncores)
    nsh = n_fine // ncores

    h0 = np.zeros((n_fine, P), np.float32)
    h0[:, 0:5] = x
    h0[:, 5:6] = sdf
    h0A = np.zeros((ncores * CHA, P), bf16)
    h0B = np.zeros((ncores * CHB, P), bf16)
    for c in range(ncores):
        sh = h0[c * nsh:(c + 1) * nsh]
        h0A[c * CHA:(c + 1) * CHA] = sh[:CHA].astype(bf16)
        h0B[c * CHB:c * CHB + (nsh - CHA)] = sh[CHA:].astype(bf16)

    xpos = x[:, :2].astype(np.float32)
    coarseT = np.ascontiguousarray(coarse_x[:, :2].T).astype(np.float32)

    in_maps = []
    for c in range(ncores):
        xx = np.zeros((2, padsh), np.float32)
        xx[:, :nsh] = xpos[c * nsh:(c + 1) * nsh].T
        xn = np.zeros((padsh, 2), np.float32)
        xn[:nsh] = xpos[c * nsh:(c + 1) * nsh]
        h0nm = np.zeros((padsh, 8), bf16)
        h0nm[:nsh, 0:6] = h0[c * nsh:(c + 1) * nsh, 0:6].astype(bf16)
        m = {
            "h0A": h0A, "h0B": h0B, "h0nm": h0nm,
            "idxA": edges[c]["idxA"], "idxB": edges[c]["idxB"],
            "sblk": edges[c]["sblk"],
            "xposT": xx, "xpos_nm": xn,
            "coarseT": coarseT, "ycoarse": coarse_y,
            "w0": np.asarray(inputs["pre_W0"], np.float32),
            "b0": np.asarray(inputs["pre_b0"], np.float32),
            "w1": np.asarray(inputs["pre_W1"], np.float32),
            "b1": np.asarray(inputs["pre_b1"], np.float32),
            "w2": np.asarray(inputs["pre_W2"], np.float32),
            "b2": np.asarray(inputs["pre_b2"], np.float32),
            "wtop": np.ascontiguousarray(np.asarray(inputs["end_W0"], np.float32)[:out_dim]),
            "we0": np.ascontiguousarray(np.asarray(inputs["end_W0"], np.float32)[out_dim:]),
            "be0": np.asarray(inputs["end_b0"], np.float32),
            "we1": np.asarray(inputs["end_W1"], np.float32),
            "be1": np.asarray(inputs["end_b1"], np.float32),
            "w5": np.asarray(inputs["end_W2"], np.float32),
            "b5": np.asarray(inputs["end_b2"], np.float32),
        }
        in_maps.append(m)
    return SLA, SLB, nt, padsh, in_maps


def run(inputs, n_fine=N_FINE, n_coarse=N_COARSE, hid=HID, out_dim=OUT,
        ncores=NCORES, sim=False, trace=False):
    SLA, SLB, nt, padsh, in_maps = _prepare(inputs, n_fine, n_coarse, hid,
                                            out_dim, ncores)
    key = (n_fine, n_coarse, hid, out_dim, ncores, tuple(SLA), tuple(SLB), nt)
    if key not in _PROGRAM_CACHE:
        _PROGRAM_CACHE[key] = build_program(n_fine, n_coarse, hid, out_dim,
                                            ncores, SLA, SLB, nt)
    nc = _PROGRAM_CACHE[key]

    nsh = n_fine // ncores
    if sim:
        from concourse.bass_interp import MultiCoreSim
        ms = MultiCoreSim(nc, ncores, num_workers=1)
        for c in range(ncores):
            for k, v in in_maps[c].items():
                ms.cores[c].tensor(k)[:] = v
        ms.simulate()
        outs = [np.array(ms.cores[c].tensor("out")) for c in range(ncores)]
        exec_ns = None
    else:
        from concourse.bass_utils import run_bass_kernel_spmd
        res = run_bass_kernel_spmd(nc, in_maps, list(range(ncores)), trace=trace)
        outs = [res.results[c]["out"] for c in range(ncores)]
        exec_ns = res.exec_time_ns

    full = np.zeros((n_fine, out_dim), np.float32)
    for c in range(ncores):
        full[c * nsh:(c + 1) * nsh] = outs[c][:nsh]
    return full, exec_ns


def kernel(**inputs):
    out, _ = run(inputs)
    return out
